# revision 2
# baseline (speedup 1.0000x reference)
"""Trainium2 Bass kernel for nn_BaselineModel_74509092651544 (CLRS-style MPNN).

Strategy (v2)
-------------
Data-parallel over graphs: 32 graphs -> 8 cores x 4 graphs.  Only the ~61k
unique (graph,src,dst) edge slots survive the masked max, so the message MLP
runs on a padded CSR slot layout.

v2 changes vs the 114us baseline:
  * All one-hot/gather matmuls use fp8 DoubleRow perf mode (0.5 cyc/col):
      - pre accumulation: planes (m1,m2 | Gsrc,Gdst) with hi/lo fp8 splits
        of m1/m2, bond term as (bw_hi,bw_lo | SOH,SOH).
      - node features: 15 DoubleRows over 3-level fp8 atom embeddings.
    Small-magnitude operands are quantized in a x256 basis (one-hot entries
    1/256, exactly representable) to dodge e4m3's subnormal floor.
  * h-matmuls batched per graph pair (ap=256 avoids the <256 f32r 4x penalty).
  * Elementwise work greedily load-balanced across ACT/DVE/GPSIMD(Pool);
    segmented max reduces (DVE-only) get an overlapping-halves "premax"
    (tensor_tensor max) on Pool when that lowers the peak engine load.
  * LayerNorm transposes run on the (idle) DMA engines via dma_start_transpose
    in bf16 -- no PE/PSUM involvement, and the LN elementwise ops become
    all-SBUF (DVE 2x/4x modes).  Stats via bn_stats/bn_aggr.  The final layer
    pools via per-graph hnorm^T @ (1/N) matmuls (no un-transpose).
  * bp2 folded into bias_h on-device (removes the per-graph msgs_used pass).
  * Compute-ordered DMA schedule; ACT tables preloaded during the DMA wait.
"""

import sys
import numpy as np

sys.path.insert(0, "/opt/trn_rl_repo")

B, N, H, L, E, OUT = 32, 128, 128, 3, 65536, 128
M = 8                 # NeuronCores
BL = B // M           # graphs per core
NEG = -1e9
EPS = 1e-5
AV, BV = 128, 16
QS = 256.0            # fp8 scale basis for small-magnitude operands

_CACHE = {}


def _f8split(arr, levels):
    """Split float array into `levels` fp8(e4m3) planes summing to ~arr."""
    import ml_dtypes
    FP8 = ml_dtypes.float8_e4m3fn
    out = []
    r = np.asarray(arr, np.float32)
    for _ in range(levels):
        q = r.astype(FP8)
        out.append(q)
        r = r - q.astype(np.float32)
    return out


# --------------------------------------------------------------------------
# Host preprocessing: integer indexing / relayout / dtype splits only.
# --------------------------------------------------------------------------

def _prep(inputs):
    import ml_dtypes
    FP8 = ml_dtypes.float8_e4m3fn
    x = np.asarray(inputs["x"]).astype(np.int64)            # [B*N, 9]
    ea = np.asarray(inputs["edge_attr"]).astype(np.int64)   # [E, 3]
    ei = np.asarray(inputs["edge_index"]).astype(np.int64)  # [2, E]

    g = ei[0] // N
    s = ei[0] % N
    d = ei[1] % N
    key = (g * N + s) * N + d
    uniq, inv = np.unique(key, return_inverse=True)
    US = uniq.size
    ug = uniq // (N * N)
    us = (uniq // N) % N
    ud = uniq % N

    # bond one-hot counts per unique slot  [US, 48]
    oh48 = np.zeros((US, 48), np.float32)
    for c in range(3):
        np.add.at(oh48, (inv, ea[:, c] + 16 * c), 1.0)

    # unique in-degree per (graph, receiver)
    deg = np.zeros((B, N), np.int64)
    np.add.at(deg, (ug, ud), 1)

    # receiver relabeling: position p holds the p-th highest-degree receiver
    rho = np.argsort(-deg, axis=1, kind="stable")        # [B, N] pos -> orig
    rho_inv = np.argsort(rho, axis=1)                    # orig -> pos
    degS = -np.sort(-deg, axis=1)                        # [B, N] desc
    Kp = np.maximum(degS.max(axis=0), 1)                 # [N]

    # group schedule (shared by all graphs/cores): (p0, R, K)
    groups = []
    p = 0
    while p < N:
        K = int(Kp[p])
        if 16 * K <= 512:
            R = 16
        elif 8 * K <= 512:
            R = 8
        else:
            R = 4
        R = min(R, N - p)
        groups.append((p, R, K))
        p += R

    def ffd(sizes):
        order_g = np.argsort(-np.asarray(sizes), kind="stable")
        tiles_used = []
        place = [None] * len(sizes)
        for gi in order_g:
            sz = sizes[gi]
            for t in range(len(tiles_used)):
                if tiles_used[t] + sz <= 512:
                    place[gi] = (t, tiles_used[t])
                    tiles_used[t] += sz
                    break
            else:
                place[gi] = (len(tiles_used), 0)
                tiles_used.append(sz)
        return place, tiles_used

    def cost(groups):
        place, tiles_used = ffd([R * K for (_, R, K) in groups])
        return (sum(tiles_used) * 1.042 + 170 * len(groups)
                + 2500 * len(tiles_used))

    # merge adjacent groups (padding the smaller K up) when it helps
    improved = True
    while improved:
        improved = False
        for i in range(len(groups) - 1):
            p0a, Ra, Ka = groups[i]
            p0b, Rb, Kb = groups[i + 1]
            if (Ra + Rb) * max(Ka, Kb) > 512:
                continue
            cand = (groups[:i] + [(p0a, Ra + Rb, max(Ka, Kb))]
                    + groups[i + 2:])
            if cost(cand) < cost(groups):
                groups = cand
                improved = True
                break

    place, tiles_used = ffd([R * K for (_, R, K) in groups])
    n_tiles = len(tiles_used)
    S_graph = 512 * n_tiles
    S_core = BL * S_graph

    # per-position lookup tables
    col_base_of_pos = np.zeros(N, np.int64)   # first column of the receiver
    K_of_pos = np.zeros(N, np.int64)
    for gi, (p0, R, K) in enumerate(groups):
        t, off = place[gi]
        for r in range(R):
            col_base_of_pos[p0 + r] = t * 512 + off + r * K
            K_of_pos[p0 + r] = K

    # slots ordered by (g, d, s): contiguous per receiver
    order = np.lexsort((us, ud, ug))
    og, od, osl = ug[order], ud[order], order
    osrc = us[order]
    recv_id = og * N + od
    first = np.concatenate([[0], np.flatnonzero(np.diff(recv_id)) + 1])
    k_rank = np.arange(len(og)) - first[np.searchsorted(recv_id[first], recv_id)]

    pos = rho_inv[og, od]
    core_r = og // BL
    col_r = (og % BL) * S_graph + col_base_of_pos[pos] + k_rank

    # padding: receivers with deg < K duplicate their first slot
    fg, fd = og[first], od[first]
    fpos = rho_inv[fg, fd]
    fdeg = deg[fg, fd]
    fK = K_of_pos[fpos]
    padc = (fK - fdeg).astype(np.int64)
    assert (padc >= 0).all()
    rep = np.repeat(np.arange(len(first)), padc)
    kpad = np.arange(len(rep)) - np.repeat(
        np.concatenate([[0], np.cumsum(padc)[:-1]]), padc
    ) + np.repeat(fdeg, padc)
    pg = fg[rep]
    core_p = pg // BL
    col_p = (pg % BL) * S_graph + col_base_of_pos[fpos[rep]] + kpad
    slot_p = osl[first][rep]
    src_p = osrc[first][rep]
    pos_p = fpos[rep]

    a_core = np.concatenate([core_r, core_p])
    a_col = np.concatenate([col_r, col_p])
    a_slot = np.concatenate([osl, slot_p])
    a_srcnew = np.concatenate([rho_inv[og, osrc], rho_inv[pg, src_p]])
    a_dstpos = np.concatenate([pos, pos_p])

    # pair-interleaved gather planes: col 2j = src one-hot, col 2j+1 = dst
    flat = a_core * S_core + a_col
    GP = np.zeros((M * S_core * 2, 128), np.float32)
    GP[2 * flat, a_srcnew] = 1.0
    GP[2 * flat + 1, a_dstpos] = 1.0
    # SOH in the 1/QS basis (counts/QS are exact in fp8); bw is scaled by QS
    SOHP = np.zeros((M * S_core * 2, 48), np.float32)
    SOHP[2 * flat] = oh48[a_slot] * (1.0 / QS)
    SOHP[2 * flat + 1] = oh48[a_slot] * (1.0 / QS)
    GP = np.ascontiguousarray(
        GP.reshape(M, 2 * S_core, 128).transpose(0, 2, 1)).astype(FP8)
    SOHP = np.ascontiguousarray(
        SOHP.reshape(M, 2 * S_core, 48).transpose(0, 2, 1)).astype(FP8)

    # atom one-hots (value 1/QS), feature-pair interleaved: [M, 5, AV, 2*BL*N]
    gg_ = np.repeat(np.arange(B), N)
    pp = np.tile(np.arange(N), B)
    orig = gg_ * N + rho[gg_, pp]                  # [B*N] column -> orig node
    BLN = BL * N
    XOHP = np.zeros((M, 5, AV, 2 * BLN), np.float32)
    mcol = np.tile(np.arange(BLN), M)
    mcore = np.repeat(np.arange(M), BLN)
    for c in range(9):
        XOHP[mcore, c // 2, x[orig, c], 2 * mcol + (c % 2)] = 1.0 / QS
    XOHP = XOHP.astype(FP8)

    # empty receivers (deg==0) -> need NEG mask path
    empt = (deg == 0)
    has_empty = bool(empt.any())
    maskrow = np.ones((M, BLN), np.float32)
    negrow = np.zeros((M, BLN), np.float32)
    if has_empty:
        eg, en = np.nonzero(empt)
        epos = rho_inv[eg, en]
        maskrow[eg // BL, (eg % BL) * N + epos] = 0.0
        negrow[eg // BL, (eg % BL) * N + epos] = NEG

    tile_used = [max(256, ((u + 7) // 8) * 8) for u in tiles_used]
    struct = dict(
        S_graph=S_graph, S_core=S_core, n_tiles=n_tiles,
        groups=[(p0, R, K, place[gi][0], place[gi][1])
                for gi, (p0, R, K) in enumerate(groups)],
        tile_used=tuple(tile_used),
        has_empty=has_empty,
    )
    percore = dict(gp=GP, sohp=SOHP, xohp=XOHP, maskrow=maskrow, negrow=negrow)
    return struct, percore


def _weight_arrays(inputs):
    f32 = np.float32
    A = {}

    Wm1 = np.asarray(inputs["Wm1"], f32)
    Wm2 = np.asarray(inputs["Wm2"], f32)
    atom = np.asarray(inputs["atom_emb"], f32)      # [9, AV, H]
    cols = []
    wmap = {}

    def add(name, arr):
        wmap[name] = (sum(c.shape[1] for c in cols), arr.shape[1])
        cols.append(np.asarray(arr, f32))

    # ---- chunk A1: preamble + layer-0 needs
    bond_T = np.zeros((128, 48), f32)
    bond_T[:, :] = np.asarray(inputs["bond_emb"], f32).reshape(48, H).T
    add("bondT", bond_T)
    add("We_0", np.asarray(inputs["We"], f32)[0])
    add("Wo2_0", np.asarray(inputs["Wo2"], f32)[0])
    add("m12_0_0", np.concatenate([Wm1[0, 0:128], Wm2[0, 0:128]], 1))
    add("Wp1_0", np.asarray(inputs["Wp1"], f32)[0])
    add("Wp2_0", np.asarray(inputs["Wp2"], f32)[0])
    add("Wo1_0_0", np.asarray(inputs["Wo1"], f32)[0, 0:128])
    ws1 = sum(c.shape[1] for c in cols)
    # ---- chunk B: layers 1-2 + head
    for l in range(1, L):
        add(f"We_{l}", np.asarray(inputs["We"], f32)[l])
        add(f"Wo2_{l}", np.asarray(inputs["Wo2"], f32)[l])
        add(f"m12_{l}_0", np.concatenate([Wm1[l, 0:128], Wm2[l, 0:128]], 1))
        add(f"m12_{l}_1", np.concatenate([Wm1[l, 128:256], Wm2[l, 128:256]], 1))
        add(f"Wp1_{l}", np.asarray(inputs["Wp1"], f32)[l])
        add(f"Wp2_{l}", np.asarray(inputs["Wp2"], f32)[l])
        add(f"Wo1_{l}_0", np.asarray(inputs["Wo1"], f32)[l, 0:128])
        add(f"Wo1_{l}_1", np.asarray(inputs["Wo1"], f32)[l, 128:256])
    add("Wh1", np.asarray(inputs["Wh1"], f32))
    add("Wh2", np.asarray(inputs["Wh2"], f32))
    add("oneN", np.full((128, 1), 1.0 / N, f32))
    A["wblob"] = np.ascontiguousarray(np.concatenate(cols, 1))
    A["_wmap"] = wmap
    A["_ws1"] = ws1

    # 3-level fp8 atom embeddings in the xQS basis, feature-pair plane layout
    at10 = np.zeros((10, AV, H), f32)
    at10[:9] = atom * QS
    blocks = []
    for p in range(5):
        lv0 = _f8split(at10[2 * p], 3)
        lv1 = _f8split(at10[2 * p + 1], 3)
        for v in range(3):
            blocks.append(np.concatenate(
                [lv0[v].astype(f32), lv1[v].astype(f32)], 1))  # [AV, 2H]
    import ml_dtypes
    A["atomp"] = np.ascontiguousarray(
        np.concatenate(blocks, 1)).astype(ml_dtypes.float8_e4m3fn)

    # identity (bf16) for the tail-layer PE transposes
    A["idnb"] = np.eye(128, dtype=f32).astype(ml_dtypes.bfloat16)

    # bias columns [128, 29]
    bc = np.zeros((H, 29), f32)
    for l in range(L):
        bc[:, 4 * l + 0] = np.asarray(inputs["bm1"], f32)[l]
        bc[:, 4 * l + 1] = np.asarray(inputs["bm2"], f32)[l]
        bc[:, 4 * l + 2] = np.asarray(inputs["be"], f32)[l]
        bc[:, 4 * l + 3] = np.asarray(inputs["bg"], f32)[l]
        bc[:, 12 + 2 * l + 0] = np.asarray(inputs["bo1"], f32)[l]
        bc[:, 12 + 2 * l + 1] = np.asarray(inputs["bo2"], f32)[l]
        bc[:, 18 + l] = np.asarray(inputs["bp1"], f32)[l]
        bc[:, 22 + l] = np.asarray(inputs["ln_s"], f32)[l]
        bc[:, 25 + l] = np.asarray(inputs["ln_b"], f32)[l]
    bc[:, 21] = EPS
    bc[:, 28] = np.asarray(inputs["bh1"], f32)
    A["bias_cols"] = bc
    A["bh2_full"] = np.ascontiguousarray(
        np.asarray(inputs["bh2"], f32).reshape(OUT, 1))
    bp2f = np.zeros((H, 4), f32)
    bp2f[:, :L] = np.asarray(inputs["bp2"], f32).T
    A["bp2f"] = bp2f
    return A


# --------------------------------------------------------------------------
# Bass program.
# --------------------------------------------------------------------------

def _build_program(struct, wmap, ws1, wcols):
    import concourse.bacc as bacc
    import concourse.mybir as mybir
    import concourse.tile as tile

    F32 = mybir.dt.float32
    S_core = struct["S_core"]

    nc = bacc.Bacc("TRN2", target_bir_lowering=False, debug=False)

    FP8 = mybir.dt.float8e4
    F32R = mybir.dt.float32r
    d = {}
    d["d_gp"] = nc.dram_tensor("gp", [128, 2 * S_core], FP8, kind="ExternalInput")
    d["d_sohp"] = nc.dram_tensor("sohp", [48, 2 * S_core], FP8, kind="ExternalInput")
    d["d_xohp"] = nc.dram_tensor("xohp", [5, AV, 2 * BL * N], FP8, kind="ExternalInput")
    d["d_atomp"] = nc.dram_tensor("atomp", [AV, 30 * H], FP8, kind="ExternalInput")
    d["d_wblob"] = nc.dram_tensor("wblob", [128, wcols], F32R, kind="ExternalInput")
    d["d_idnb"] = nc.dram_tensor("idnb", [128, 128], mybir.dt.bfloat16,
                                 kind="ExternalInput")
    d["d_bc"] = nc.dram_tensor("bias_cols", [H, 29], F32, kind="ExternalInput")
    d["d_bh2"] = nc.dram_tensor("bh2_full", [OUT, 1], F32, kind="ExternalInput")
    d["d_bp2f"] = nc.dram_tensor("bp2f", [H, 4], F32R, kind="ExternalInput")
    if struct["has_empty"]:
        d["d_mask"] = nc.dram_tensor("maskrow", [1, BL * N], F32, kind="ExternalInput")
        d["d_neg"] = nc.dram_tensor("negrow", [1, BL * N], F32, kind="ExternalInput")
    d["d_out"] = nc.dram_tensor("out", [OUT, BL], F32, kind="ExternalOutput")

    with tile.TileContext(nc) as tc:
        _emit(tc, nc, d, struct, wmap, ws1, mybir)
    nc.compile()
    return nc


def _emit(tc, nc, d, struct, wmap, ws1, mybir):
    import contextlib
    ctx = contextlib.ExitStack()
    F32 = mybir.dt.float32
    F32R = mybir.dt.float32r
    BF16 = mybir.dt.bfloat16
    FP8 = mybir.dt.float8e4
    AF = mybir.ActivationFunctionType
    ALU = mybir.AluOpType
    AX = mybir.AxisListType
    DR = mybir.MatmulPerfMode.DoubleRow

    S_graph = struct["S_graph"]
    S_core = struct["S_core"]
    groups = struct["groups"]
    tile_used = struct["tile_used"]
    has_empty = struct["has_empty"]
    n_tiles = struct["n_tiles"]

    # ---- engine load balancer -------------------------------------------
    load = {"ACT": 0.0, "DVE": 0.0, "POOL": 0.0}
    OVH = {"ACT": 215.0, "DVE": 170.0, "POOL": 130.0}
    ENG = {"ACT": nc.scalar, "DVE": nc.vector, "POOL": nc.gpsimd}

    def rate(e, sbuf=False, b2=False):
        if e == "ACT":
            return 0.833
        if e == "POOL":
            return 1.389
        if sbuf and b2:
            return 0.26
        if sbuf:
            return 0.521
        return 1.042

    def pick(cands, cols, sbuf=False, b2=False):
        e = min(cands,
                key=lambda e: load[e] + cols * rate(e, sbuf, b2) + OVH[e])
        load[e] += cols * rate(e, sbuf, b2) + OVH[e]
        return e

    def charge(e, cols):
        load[e] += cols * rate(e) + OVH[e]

    def ew_relu(out, in_, bias_ap, cols, cands=("ACT", "DVE", "POOL")):
        e = pick(cands, cols)
        if e == "ACT":
            nc.scalar.activation(out, in_, AF.Relu, bias=bias_ap)
        else:
            ENG[e].tensor_scalar(out, in_, bias_ap, 0.0,
                                 op0=ALU.add, op1=ALU.max)

    def ew_copy(out, in_, cols, cands=("ACT", "DVE", "POOL"), scale=None):
        e = pick(cands, cols)
        if e == "ACT":
            if scale is None:
                nc.scalar.activation(out, in_, AF.Copy)
            else:
                nc.scalar.activation(out, in_, AF.Copy, scale=scale)
        elif scale is None:
            ENG[e].tensor_copy(out, in_)
        else:
            ENG[e].tensor_scalar(out, in_, scale, None, op0=ALU.mult)

    def ew_stt(out, in0, scalar, in1, op0, op1, cols, cands=("DVE", "POOL")):
        e = pick(cands, cols)
        ENG[e].scalar_tensor_tensor(out, in0, scalar, in1, op0=op0, op1=op1)

    def ew_ts2(out, in_, s1, s2, op0, op1, cols, cands=("DVE", "POOL"),
               sbuf=False, b2=False):
        e = pick(cands, cols, sbuf, b2)
        ENG[e].tensor_scalar(out, in_, s1, s2, op0=op0, op1=op1)

    def ew_scale_bias(out, in_, s_ap, b_ap, cols, cands=("ACT", "DVE", "POOL"),
                      sbuf=False, b2=False):
        e = pick(cands, cols, sbuf, b2)
        if e == "ACT":
            nc.scalar.activation(out, in_, AF.Identity, scale=s_ap, bias=b_ap)
        else:
            ENG[e].tensor_scalar(out, in_, s_ap, b_ap,
                                 op0=ALU.mult, op1=ALU.add)

    # ---- pools -----------------------------------------------------------
    pG = ctx.enter_context(tc.tile_pool(name="pG", bufs=1))
    pW = ctx.enter_context(tc.tile_pool(name="pW", bufs=1))
    pAct = ctx.enter_context(tc.tile_pool(name="pAct", bufs=4))
    pNM = ctx.enter_context(tc.tile_pool(name="pNM", bufs=1))
    pMB = ctx.enter_context(tc.tile_pool(name="pMB", bufs=2))
    pLN = ctx.enter_context(tc.tile_pool(name="pLN", bufs=2))
    ps_pre = ctx.enter_context(tc.tile_pool(name="ps_pre", bufs=2, space="PSUM"))
    ps_p1 = ctx.enter_context(tc.tile_pool(name="ps_p1", bufs=2, space="PSUM"))
    ps_p2 = ctx.enter_context(tc.tile_pool(name="ps_p2", bufs=2, space="PSUM"))
    ps_misc = ctx.enter_context(tc.tile_pool(name="ps_misc", bufs=2, space="PSUM"))

    def mps(name, dt=F32):
        return ps_misc.tile([128, 512], dt, name=name, tag="mps")

    # ---- ACT table preload + PE p-state warmup (during the DMA wait) -----
    dummy = pW.tile([1, 1], F32, name="dummy")
    nc.gpsimd.memset(dummy[:], 1.0)
    for fn in (AF.Relu, AF.Identity, AF.Sqrt, AF.Copy):
        nc.scalar.activation(dummy[:], dummy[:], fn)
    dummyr = dummy[:].bitcast(F32R)
    warm = ps_misc.tile([128, 512], F32, name="warm", tag="mps")
    for _ in range(12):
        nc.tensor.matmul(warm[0:1, 0:1], dummyr, dummyr,
                         start=True, stop=True)

    # ---- resident tiles + DMA schedule ----------------------------------
    atomp_sb = pW.tile([AV, 30 * H], FP8, name="atomp_sb")
    nc.sync.dma_start(atomp_sb[:], d["d_atomp"].ap())
    xohp_sb = pW.tile([AV, 10 * BL * N], FP8, name="xohp_sb")
    XW = 2 * BL * N
    nc.sync.dma_start(xohp_sb[:, 0:XW], d["d_xohp"].ap()[0])
    for p in range(1, 5):
        nc.sync.dma_start(xohp_sb[:, p * XW:(p + 1) * XW],
                          d["d_xohp"].ap()[p])
    gp_sb = pG.tile([128, 2 * S_core], FP8, name="gp_sb")
    sohp_sb = pG.tile([48, 2 * S_core], FP8, name="sohp_sb")
    SG2 = 2 * S_graph
    wcols = sum(w for (_, w) in wmap.values())
    wblob_sb = pW.tile([128, wcols], F32R, name="wblob_sb")
    nc.sync.dma_start(wblob_sb[:, 0:ws1], d["d_wblob"].ap()[:, 0:ws1])
    nc.sync.dma_start(gp_sb[:, 0:SG2], d["d_gp"].ap()[:, 0:SG2])
    nc.sync.dma_start(sohp_sb[:, 0:SG2], d["d_sohp"].ap()[:, 0:SG2])
    bc_sb = pW.tile([H, 29], F32, name="bc_sb")
    nc.sync.dma_start(bc_sb[:], d["d_bc"].ap())
    bp2f_sb = pW.tile([H, 4], F32R, name="bp2f_sb")
    nc.sync.dma_start(bp2f_sb[:], d["d_bp2f"].ap())
    for g in range(1, BL):
        sl = slice(g * SG2, (g + 1) * SG2)
        nc.sync.dma_start(gp_sb[:, sl], d["d_gp"].ap()[:, sl])
        nc.sync.dma_start(sohp_sb[:, sl], d["d_sohp"].ap()[:, sl])
    bh2_sb = pW.tile([OUT, 1], F32, name="bh2_sb")
    nc.sync.dma_start(bh2_sb[:], d["d_bh2"].ap())
    idnb_sb = pW.tile([128, 128], mybir.dt.bfloat16, name="idnb_sb")
    nc.sync.dma_start(idnb_sb[:], d["d_idnb"].ap())
    nc.sync.dma_start(wblob_sb[:, ws1:], d["d_wblob"].ap()[:, ws1:])

    if has_empty:
        mrow_sb = pW.tile([1, BL * N], F32, name="mrow_sb")
        nc.sync.dma_start(mrow_sb[:], d["d_mask"].ap())
        nrow_sb = pW.tile([1, BL * N], F32, name="nrow_sb")
        nc.sync.dma_start(nrow_sb[:], d["d_neg"].ap())
        mask_bc = pW.tile([128, BL * N], F32, name="mask_bc")
        nc.gpsimd.partition_broadcast(mask_bc[:], mrow_sb[:])
        neg_bc = pW.tile([128, BL * N], F32, name="neg_bc")
        nc.gpsimd.partition_broadcast(neg_bc[:], nrow_sb[:])

    def W(name):
        off, w = wmap[name]
        return wblob_sb[:, off:off + w]

    def pair(ap):
        return ap.rearrange("p (two h) -> p two h", two=2)

    def mov_pair(ap):
        return ap.rearrange("p (w two) -> p two w", two=2)

    # ---- preamble compute (overlaps G DMAs) ------------------------------
    bias_pre = pW.tile([128, L], F32, name="bias_pre")
    nc.vector.tensor_reduce(
        bias_pre[:], bc_sb[:, 0:4 * L].rearrange("p (l f) -> p l f", l=L),
        axis=AX.X, op=ALU.add)
    bo12 = pW.tile([128, L], F32, name="bo12")
    nc.vector.tensor_reduce(
        bo12[:], bc_sb[:, 12:12 + 2 * L].rearrange("p (l f) -> p l f", l=L),
        axis=AX.X, op=ALU.add)

    # node features: 15 fp8 DoubleRow matmuls (5 pairs x 3 levels)
    nf_ps = mps("nf_ps")
    for p in range(5):
        xs = mov_pair(xohp_sb[:, p * XW:(p + 1) * XW])
        for v in range(3):
            blk = (p * 3 + v) * 2 * H
            nc.tensor.matmul(nf_ps[:, 0:BL * N],
                             pair(atomp_sb[:, blk:blk + 2 * H]), xs,
                             start=(p == 0 and v == 0),
                             stop=(p == 4 and v == 2), perf_mode=DR)
    nf = pNM.tile([128, BL * N], BF16, name="nf")
    nc.scalar.activation(nf[:], nf_ps[:, 0:BL * N], AF.Copy)

    def prep_layer(l):
        """bw (bond @ We, xQS basis) + bias_h for layer l; emitted right
        before the layer so chunk-B weight DMAs never block the PE stream."""
        bw_ps = mps("bw_ps")
        nc.tensor.matmul(bw_ps[0:48, 0:H], W("bondT"), W(f"We_{l}"),
                         start=True, stop=True)
        bwp = pW.tile([48, 2 * H], FP8, name=f"bwp{l}")
        nc.scalar.activation(bwp[:, 0:H], bw_ps[0:48, 0:H], AF.Copy, scale=QS)
        nc.vector.scalar_tensor_tensor(bwp[:, H:2 * H], bw_ps[0:48, 0:H], QS,
                                       bwp[:, 0:H], op0=ALU.mult,
                                       op1=ALU.subtract)
        bwp_l[l] = bwp

        bh_ps = mps("bh_ps")
        nc.tensor.matmul(bh_ps[:, 0:2], W(f"Wo2_{l}"),
                         bp2f_sb[:, l:l + 2], start=True, stop=True)
        bias_h = pW.tile([128, 1], F32, name=f"bias_h{l}")
        nc.vector.tensor_tensor(bias_h[:], bh_ps[:, 0:1], bo12[:, l:l + 1],
                                op=ALU.add)
        bias_h_l[l] = bias_h

    bwp_l, bias_h_l = {}, {}
    prep_layer(0)

    # ---- layers (software-pipelined emission) ----------------------------
    ST = {}

    def ensure_state(l):
        if l in ST:
            return
        st = {}
        st["msgs_max"] = pLN.tile([128, BL * N], BF16, name=f"msgs_max{l}",
                                  tag="msgs_max", bufs=2)
        st["h_fm"] = pLN.tile([128, BL * N], BF16, name=f"h_fm{l}",
                              tag="h_fm", bufs=2)
        if l < L - 1:
            st["hid"] = pNM.tile([128, BL * N], BF16, name=f"hid{l}",
                                 tag=f"hid{l}")
        else:
            st["ge_sb"] = pLN.tile([128, BL], F32R, name="ge_sb", tag="ge_sb")
        ST[l] = st

    M12P = {}

    def do_m12(l, gg):
        ensure_state(l)
        gsl = slice(gg * N, (gg + 1) * N)
        ps_m = mps("ps_m")
        nc.tensor.matmul(ps_m[:, 0:2 * H], nf[:, gsl], W(f"m12_{l}_0"),
                         start=True, stop=(l == 0))
        if l > 0:
            nc.tensor.matmul(ps_m[:, 0:2 * H], ST[l - 1]["hid"][:, gsl],
                             W(f"m12_{l}_1"), start=False, stop=True)
        m12h = pMB.tile([128, 2 * H], FP8, name=f"m12h{gg}", tag=f"m12h{gg}")
        ew_copy(m12h[:], ps_m[:, 0:2 * H], 2 * H)
        M12P[(l, gg)] = pair(m12h[:])

    def do_tiles(l, gg, t0, t1):
        msgs_max = ST[l]["msgs_max"]
        m12h_p = M12P[(l, gg)]
        bwp_pair = pair(bwp_l[l][:])
        for t in range(t0, t1):
            w = tile_used[t]
            c0 = 2 * (gg * S_graph + t * 512)
            gps = mov_pair(gp_sb[:, c0:c0 + 2 * w])
            sps = mov_pair(sohp_sb[:, c0:c0 + 2 * w])
            pre = ps_pre.tile([128, 512], F32, name="pre")
            nc.tensor.matmul(pre[:, 0:w], m12h_p, gps,
                             start=True, stop=False, perf_mode=DR)
            nc.tensor.matmul(pre[:, 0:w], bwp_pair, sps,
                             start=False, stop=True, perf_mode=DR)
            msgs1 = pAct.tile([128, 512], F32R, name="msgs1", tag="msgs1")
            ew_relu(msgs1[:, 0:w], pre[:, 0:w], bias_pre[:, l:l + 1], w)
            p1 = ps_p1.tile([128, 512], F32, name="p1")
            nc.tensor.matmul(p1[:, 0:w], W(f"Wp1_{l}"), msgs1[:, 0:w],
                             start=True, stop=True)
            msgs2 = pAct.tile([128, 512], F32R, name="msgs2", tag="msgs2")
            ew_relu(msgs2[:, 0:w], p1[:, 0:w], bc_sb[:, 18 + l:19 + l], w)
            p2 = ps_p2.tile([128, 512], F32, name="p2")
            nc.tensor.matmul(p2[:, 0:w], W(f"Wp2_{l}"), msgs2[:, 0:w],
                             start=True, stop=True)
            for (p0, R, K, gt, off) in groups:
                if gt != t:
                    continue
                out_ap = msgs_max[:, gg * N + p0: gg * N + p0 + R]
                seg = p2[:, off:off + R * K].rearrange("p (r k) -> p r k", r=R)
                nc.vector.tensor_reduce(out_ap, seg, axis=AX.X, op=ALU.max)
                load["DVE"] += R * K * 1.042 + OVH["DVE"]

    def do_h(l, gg):
        msgs_max, h_fm = ST[l]["msgs_max"], ST[l]["h_fm"]
        gsl = slice(gg * N, (gg + 1) * N)
        if has_empty:
            mm = pLN.tile([128, N], F32, name="mmx", tag="mmx", bufs=4)
            nc.vector.tensor_tensor(mm[:], msgs_max[:, gsl],
                                    mask_bc[:, gsl], op=ALU.mult)
            nc.vector.tensor_tensor(msgs_max[:, gsl], mm[:],
                                    neg_bc[:, gsl], op=ALU.add)
        h_ps = mps("h_ps")
        nc.tensor.matmul(h_ps[:, 0:N], W(f"Wo1_{l}_0"), nf[:, gsl],
                         start=True, stop=False)
        if l > 0:
            nc.tensor.matmul(h_ps[:, 0:N], W(f"Wo1_{l}_1"),
                             ST[l - 1]["hid"][:, gsl], start=False, stop=False)
        nc.tensor.matmul(h_ps[:, 0:N], W(f"Wo2_{l}"), msgs_max[:, gsl],
                         start=False, stop=True)
        ew_relu(h_fm[:, gsl], h_ps[:, 0:N], bias_h_l[l][:], N)

    def do_ln(l, gg):
        h_fm = ST[l]["h_fm"]
        gsl = slice(gg * N, (gg + 1) * N)
        if l < L - 1:
            # DMA-engine transpose: no PE stall, latency hidden by tiles
            hn = pLN.tile([128, 128], BF16, name="hn", tag="hn", bufs=4)
            nc.sync.dma_start_transpose(hn[:], h_fm[:, gsl])
        else:
            # tail: PE transpose (53ns) keeps the closing chain short
            tp = mps("tp_ps", BF16)
            nc.tensor.transpose(tp[:, 0:128], h_fm[:, gsl], idnb_sb[:])
            hn = tp[:, 0:128]
        st6 = pLN.tile([128, 6], F32, name="st6", tag="st6", bufs=4)
        nc.vector.bn_stats(st6[:], hn)
        charge("DVE", 128)
        st2 = pLN.tile([128, 2], F32, name="st2", tag="st2", bufs=4)
        nc.vector.bn_aggr(st2[:], st6[:])
        charge("DVE", 8)
        std = pLN.tile([128, 1], F32, name="std", tag="std", bufs=4)
        nc.scalar.activation(std[:], st2[:, 1:2], AF.Sqrt,
                             bias=bc_sb[:, 21:22])
        charge("ACT", 1)
        rstd = pLN.tile([128, 1], F32, name="rstd", tag="rstd", bufs=4)
        nc.vector.reciprocal(rstd[:], std[:])
        charge("DVE", 1)
        hnorm = pLN.tile([128, 128], BF16, name="hnorm", tag="hnorm", bufs=4)
        ln_cands = ("DVE", "POOL") if l < L - 1 else ("DVE",)
        ew_ts2(hnorm[:], hn, st2[:, 0:1], rstd[:],
               ALU.subtract, ALU.mult, 128, cands=ln_cands,
               sbuf=(l < L - 1), b2=(l < L - 1))
        if l < L - 1:
            hb = pLN.tile([128, 128], BF16, name="hb", tag="hb", bufs=4)
            nc.sync.dma_start_transpose(hb[:], hnorm[:])
            ew_scale_bias(ST[l]["hid"][:, gsl], hb[:],
                          bc_sb[:, 22 + l:23 + l], bc_sb[:, 25 + l:26 + l],
                          128, sbuf=True)
        else:
            ge_g = mps(f"ge_ps{gg}")
            nc.tensor.matmul(ge_g[:, 0:1], hnorm[:], W("oneN"),
                             start=True, stop=True)
            nc.scalar.activation(ST[l]["ge_sb"][:, gg:gg + 1], ge_g[:, 0:1],
                                 AF.Identity, scale=bc_sb[:, 24:25],
                                 bias=bc_sb[:, 27:28])
            charge("ACT", 1)

    # pipelined schedule: per-graph h/LN chains are emitted mid-way through
    # the NEXT graph's tile stream (so the PE never waits on trailing
    # reduces), and layer l+1's first graphs overlap layer l's close.
    nsplit = min(3, n_tiles)
    for l in range(L):
        for i in range(BL):
            do_m12(l, i)
            do_tiles(l, i, 0, nsplit)
            if i == 0:
                if l > 0:
                    do_h(l - 1, 3)
                    do_ln(l - 1, 3)
                elif n_tiles > nsplit:
                    pass
            else:
                do_h(l, i - 1)
                do_ln(l, i - 1)
            if i == 2 and l < L - 1:
                prep_layer(l + 1)
            do_tiles(l, i, nsplit, n_tiles)
    do_h(L - 1, 3)
    do_ln(L - 1, 3)
    ge_sb = ST[L - 1]["ge_sb"]

    # ---- head ------------------------------------------------------------
    o1 = mps("o1_ps")
    nc.tensor.matmul(o1[:, 0:BL], W("Wh1"), ge_sb[:], start=True, stop=True)
    t1 = pLN.tile([128, BL], F32R, name="t1", tag="t1")
    nc.scalar.activation(t1[:], o1[:, 0:BL], AF.Relu, bias=bc_sb[:, 28:29])
    o2 = mps("o2_ps")
    nc.tensor.matmul(o2[:, 0:BL], W("Wh2"), t1[:], start=True, stop=True)
    out_sb = pLN.tile([OUT, BL], F32, name="out_sb", tag="out_sb")
    nc.scalar.activation(out_sb[:], o2[:, 0:BL], AF.Identity, bias=bh2_sb[:])
    nc.sync.dma_start(d["d_out"].ap(), out_sb[:])
    ctx.close()


# --------------------------------------------------------------------------
# Entry point.
# --------------------------------------------------------------------------

def build(inputs):
    struct, percore = _prep(inputs)
    A = _weight_arrays(inputs)
    wmap = A.pop("_wmap")
    ws1 = A.pop("_ws1")
    key = (struct["S_graph"], struct["n_tiles"],
           tuple(struct["groups"]), struct["has_empty"])
    if key not in _CACHE:
        _CACHE[key] = _build_program(struct, wmap, ws1, A["wblob"].shape[1])
    nc = _CACHE[key]

    in_maps = []
    for c in range(M):
        im = dict(
            gp=percore["gp"][c], sohp=percore["sohp"][c],
            xohp=percore["xohp"][c],
        )
        if struct["has_empty"]:
            im["maskrow"] = percore["maskrow"][c:c + 1]
            im["negrow"] = percore["negrow"][c:c + 1]
        for k, v in A.items():
            im[k] = v
        in_maps.append(im)
    return nc, in_maps, struct


def kernel(**inputs):
    from concourse import bass_utils
    nc, in_maps, struct = build(inputs)
    res = bass_utils.run_bass_kernel_spmd(nc, in_maps, core_ids=list(range(M)))
    out = np.zeros((B, OUT), np.float32)
    for c in range(M):
        out[c * BL:(c + 1) * BL] = res.results[c]["out"].T
    return out


# revision 3
# speedup vs baseline: 1.0321x; 1.0321x over previous
"""Trainium2 Bass kernel for nn_BaselineModel_74509092651544 (CLRS-style MPNN).

Strategy (v2)
-------------
Data-parallel over graphs: 32 graphs -> 8 cores x 4 graphs.  Only the ~61k
unique (graph,src,dst) edge slots survive the masked max, so the message MLP
runs on a padded CSR slot layout.

v2 changes vs the 114us baseline:
  * All one-hot/gather matmuls use fp8 DoubleRow perf mode (0.5 cyc/col):
      - pre accumulation: planes (m1,m2 | Gsrc,Gdst) with hi/lo fp8 splits
        of m1/m2, bond term as (bw_hi,bw_lo | SOH,SOH).
      - node features: 15 DoubleRows over 3-level fp8 atom embeddings.
    Small-magnitude operands are quantized in a x256 basis (one-hot entries
    1/256, exactly representable) to dodge e4m3's subnormal floor.
  * h-matmuls batched per graph pair (ap=256 avoids the <256 f32r 4x penalty).
  * Elementwise work greedily load-balanced across ACT/DVE/GPSIMD(Pool);
    segmented max reduces (DVE-only) get an overlapping-halves "premax"
    (tensor_tensor max) on Pool when that lowers the peak engine load.
  * LayerNorm transposes run on the (idle) DMA engines via dma_start_transpose
    in bf16 -- no PE/PSUM involvement, and the LN elementwise ops become
    all-SBUF (DVE 2x/4x modes).  Stats via bn_stats/bn_aggr.  The final layer
    pools via per-graph hnorm^T @ (1/N) matmuls (no un-transpose).
  * bp2 folded into bias_h on-device (removes the per-graph msgs_used pass).
  * Compute-ordered DMA schedule; ACT tables preloaded during the DMA wait.
"""

import sys
import numpy as np

sys.path.insert(0, "/opt/trn_rl_repo")

B, N, H, L, E, OUT = 32, 128, 128, 3, 65536, 128
M = 8                 # NeuronCores
BL = B // M           # graphs per core
NEG = -1e9
EPS = 1e-5
AV, BV = 128, 16
QS = 256.0            # fp8 scale basis for small-magnitude operands

_CACHE = {}


def _f8split(arr, levels):
    """Split float array into `levels` fp8(e4m3) planes summing to ~arr."""
    import ml_dtypes
    FP8 = ml_dtypes.float8_e4m3fn
    out = []
    r = np.asarray(arr, np.float32)
    for _ in range(levels):
        q = r.astype(FP8)
        out.append(q)
        r = r - q.astype(np.float32)
    return out


# --------------------------------------------------------------------------
# Host preprocessing: integer indexing / relayout / dtype splits only.
# --------------------------------------------------------------------------

def _prep(inputs):
    import ml_dtypes
    FP8 = ml_dtypes.float8_e4m3fn
    x = np.asarray(inputs["x"]).astype(np.int64)            # [B*N, 9]
    ea = np.asarray(inputs["edge_attr"]).astype(np.int64)   # [E, 3]
    ei = np.asarray(inputs["edge_index"]).astype(np.int64)  # [2, E]

    g = ei[0] // N
    s = ei[0] % N
    d = ei[1] % N
    key = (g * N + s) * N + d
    uniq, inv = np.unique(key, return_inverse=True)
    US = uniq.size
    ug = uniq // (N * N)
    us = (uniq // N) % N
    ud = uniq % N

    # bond one-hot counts per unique slot  [US, 48]
    oh48 = np.zeros((US, 48), np.float32)
    for c in range(3):
        np.add.at(oh48, (inv, ea[:, c] + 16 * c), 1.0)

    # unique in-degree per (graph, receiver)
    deg = np.zeros((B, N), np.int64)
    np.add.at(deg, (ug, ud), 1)

    # receiver relabeling: position p holds the p-th highest-degree receiver
    rho = np.argsort(-deg, axis=1, kind="stable")        # [B, N] pos -> orig
    rho_inv = np.argsort(rho, axis=1)                    # orig -> pos
    degS = -np.sort(-deg, axis=1)                        # [B, N] desc
    Kp = np.maximum(degS.max(axis=0), 1)                 # [N]

    # group schedule (shared by all graphs/cores): (p0, R, K)
    groups = []
    p = 0
    while p < N:
        K = int(Kp[p])
        if 16 * K <= 512:
            R = 16
        elif 8 * K <= 512:
            R = 8
        else:
            R = 4
        R = min(R, N - p)
        groups.append((p, R, K))
        p += R

    def ffd(sizes):
        order_g = np.argsort(-np.asarray(sizes), kind="stable")
        tiles_used = []
        place = [None] * len(sizes)
        for gi in order_g:
            sz = sizes[gi]
            for t in range(len(tiles_used)):
                if tiles_used[t] + sz <= 512:
                    place[gi] = (t, tiles_used[t])
                    tiles_used[t] += sz
                    break
            else:
                place[gi] = (len(tiles_used), 0)
                tiles_used.append(sz)
        return place, tiles_used

    def cost(groups):
        place, tiles_used = ffd([R * K for (_, R, K) in groups])
        return (sum(tiles_used) * 1.042 + 170 * len(groups)
                + 2500 * len(tiles_used))

    # merge adjacent groups (padding the smaller K up) when it helps
    improved = True
    while improved:
        improved = False
        for i in range(len(groups) - 1):
            p0a, Ra, Ka = groups[i]
            p0b, Rb, Kb = groups[i + 1]
            if (Ra + Rb) * max(Ka, Kb) > 512:
                continue
            cand = (groups[:i] + [(p0a, Ra + Rb, max(Ka, Kb))]
                    + groups[i + 2:])
            if cost(cand) < cost(groups):
                groups = cand
                improved = True
                break

    place, tiles_used = ffd([R * K for (_, R, K) in groups])
    n_tiles = len(tiles_used)
    S_graph = 512 * n_tiles
    S_core = BL * S_graph

    # per-position lookup tables
    col_base_of_pos = np.zeros(N, np.int64)   # first column of the receiver
    K_of_pos = np.zeros(N, np.int64)
    for gi, (p0, R, K) in enumerate(groups):
        t, off = place[gi]
        for r in range(R):
            col_base_of_pos[p0 + r] = t * 512 + off + r * K
            K_of_pos[p0 + r] = K

    # slots ordered by (g, d, s): contiguous per receiver
    order = np.lexsort((us, ud, ug))
    og, od, osl = ug[order], ud[order], order
    osrc = us[order]
    recv_id = og * N + od
    first = np.concatenate([[0], np.flatnonzero(np.diff(recv_id)) + 1])
    k_rank = np.arange(len(og)) - first[np.searchsorted(recv_id[first], recv_id)]

    pos = rho_inv[og, od]
    core_r = og // BL
    col_r = (og % BL) * S_graph + col_base_of_pos[pos] + k_rank

    # padding: receivers with deg < K duplicate their first slot
    fg, fd = og[first], od[first]
    fpos = rho_inv[fg, fd]
    fdeg = deg[fg, fd]
    fK = K_of_pos[fpos]
    padc = (fK - fdeg).astype(np.int64)
    assert (padc >= 0).all()
    rep = np.repeat(np.arange(len(first)), padc)
    kpad = np.arange(len(rep)) - np.repeat(
        np.concatenate([[0], np.cumsum(padc)[:-1]]), padc
    ) + np.repeat(fdeg, padc)
    pg = fg[rep]
    core_p = pg // BL
    col_p = (pg % BL) * S_graph + col_base_of_pos[fpos[rep]] + kpad
    slot_p = osl[first][rep]
    src_p = osrc[first][rep]
    pos_p = fpos[rep]

    a_core = np.concatenate([core_r, core_p])
    a_col = np.concatenate([col_r, col_p])
    a_slot = np.concatenate([osl, slot_p])
    a_srcnew = np.concatenate([rho_inv[og, osrc], rho_inv[pg, src_p]])
    a_dstpos = np.concatenate([pos, pos_p])

    # pair-interleaved gather planes: col 2j = src one-hot, col 2j+1 = dst
    flat = a_core * S_core + a_col
    GP = np.zeros((M * S_core * 2, 128), np.float32)
    GP[2 * flat, a_srcnew] = 1.0
    GP[2 * flat + 1, a_dstpos] = 1.0
    # SOH in the 1/QS basis (counts/QS are exact in fp8); bw is scaled by QS
    SOHP = np.zeros((M * S_core * 2, 48), np.float32)
    SOHP[2 * flat] = oh48[a_slot] * (1.0 / QS)
    SOHP[2 * flat + 1] = oh48[a_slot] * (1.0 / QS)
    GP = np.ascontiguousarray(
        GP.reshape(M, 2 * S_core, 128).transpose(0, 2, 1)).astype(FP8)
    SOHP = np.ascontiguousarray(
        SOHP.reshape(M, 2 * S_core, 48).transpose(0, 2, 1)).astype(FP8)

    # atom one-hots (value 1/QS), feature-pair interleaved: [M, 5, AV, 2*BL*N]
    gg_ = np.repeat(np.arange(B), N)
    pp = np.tile(np.arange(N), B)
    orig = gg_ * N + rho[gg_, pp]                  # [B*N] column -> orig node
    BLN = BL * N
    XOHP = np.zeros((M, 5, AV, 2 * BLN), np.float32)
    mcol = np.tile(np.arange(BLN), M)
    mcore = np.repeat(np.arange(M), BLN)
    for c in range(9):
        XOHP[mcore, c // 2, x[orig, c], 2 * mcol + (c % 2)] = 1.0 / QS
    XOHP = XOHP.astype(FP8)

    # empty receivers (deg==0) -> need NEG mask path
    empt = (deg == 0)
    has_empty = bool(empt.any())
    maskrow = np.ones((M, BLN), np.float32)
    negrow = np.zeros((M, BLN), np.float32)
    if has_empty:
        eg, en = np.nonzero(empt)
        epos = rho_inv[eg, en]
        maskrow[eg // BL, (eg % BL) * N + epos] = 0.0
        negrow[eg // BL, (eg % BL) * N + epos] = NEG

    tile_used = [max(256, ((u + 7) // 8) * 8) for u in tiles_used]
    struct = dict(
        S_graph=S_graph, S_core=S_core, n_tiles=n_tiles,
        groups=[(p0, R, K, place[gi][0], place[gi][1])
                for gi, (p0, R, K) in enumerate(groups)],
        tile_used=tuple(tile_used),
        has_empty=has_empty,
    )
    percore = dict(gp=GP, sohp=SOHP, xohp=XOHP, maskrow=maskrow, negrow=negrow)
    return struct, percore


def _weight_arrays(inputs):
    f32 = np.float32
    A = {}

    import ml_dtypes
    Wm1 = np.asarray(inputs["Wm1"], f32)
    Wm2 = np.asarray(inputs["Wm2"], f32)
    atom = np.asarray(inputs["atom_emb"], f32)      # [9, AV, H]
    cols = []
    wmap = {}
    bcols = []
    bmap = {}

    def add(name, arr):
        wmap[name] = (sum(c.shape[1] for c in cols), arr.shape[1])
        cols.append(np.asarray(arr, f32))

    def addb(name, arr):
        bmap[name] = (sum(c.shape[1] for c in bcols), arr.shape[1])
        bcols.append(np.asarray(arr, f32))

    # f32r blob (operands of f32r matmuls); chunk A = preamble + layer 0
    bond_T = np.zeros((128, 48), f32)
    bond_T[:, :] = np.asarray(inputs["bond_emb"], f32).reshape(48, H).T
    add("bondT", bond_T)
    add("We_0", np.asarray(inputs["We"], f32)[0])
    add("Wp1_0", np.asarray(inputs["Wp1"], f32)[0])
    add("Wp2_0", np.asarray(inputs["Wp2"], f32)[0])
    ws1 = sum(c.shape[1] for c in cols)
    for l in range(1, L):
        add(f"We_{l}", np.asarray(inputs["We"], f32)[l])
        add(f"Wp1_{l}", np.asarray(inputs["Wp1"], f32)[l])
        add(f"Wp2_{l}", np.asarray(inputs["Wp2"], f32)[l])
    A["wblob"] = np.ascontiguousarray(np.concatenate(cols, 1))

    # bf16 blob (operands of bf16 matmuls); chunk A = layer 0
    addb("m12_0_0", np.concatenate([Wm1[0, 0:128], Wm2[0, 0:128]], 1))
    addb("Wo1_0_0", np.asarray(inputs["Wo1"], f32)[0, 0:128])
    addb("Wo2_0", np.asarray(inputs["Wo2"], f32)[0])
    bp2f = np.zeros((H, 4), f32)
    bp2f[:, :L] = np.asarray(inputs["bp2"], f32).T
    addb("bp2fb", bp2f)
    bs1 = sum(c.shape[1] for c in bcols)
    for l in range(1, L):
        addb(f"m12_{l}_0", np.concatenate([Wm1[l, 0:128], Wm2[l, 0:128]], 1))
        addb(f"m12_{l}_1", np.concatenate([Wm1[l, 128:256], Wm2[l, 128:256]], 1))
        addb(f"Wo1_{l}_0", np.asarray(inputs["Wo1"], f32)[l, 0:128])
        addb(f"Wo1_{l}_1", np.asarray(inputs["Wo1"], f32)[l, 128:256])
        addb(f"Wo2_{l}", np.asarray(inputs["Wo2"], f32)[l])
    addb("Wh1", np.asarray(inputs["Wh1"], f32))
    addb("Wh2", np.asarray(inputs["Wh2"], f32))
    addb("oneN", np.full((128, 1), 1.0 / N, f32))
    A["wbb"] = np.ascontiguousarray(
        np.concatenate(bcols, 1)).astype(ml_dtypes.bfloat16)
    A["_wmap"] = wmap
    A["_bmap"] = bmap
    A["_ws1"] = ws1
    A["_bs1"] = bs1

    # 3-level fp8 atom embeddings in the xQS basis, feature-pair plane layout
    at10 = np.zeros((10, AV, H), f32)
    at10[:9] = atom * QS
    blocks = []
    for p in range(5):
        lv0 = _f8split(at10[2 * p], 3)
        lv1 = _f8split(at10[2 * p + 1], 3)
        for v in range(3):
            blocks.append(np.concatenate(
                [lv0[v].astype(f32), lv1[v].astype(f32)], 1))  # [AV, 2H]
    import ml_dtypes
    A["atomp"] = np.ascontiguousarray(
        np.concatenate(blocks, 1)).astype(ml_dtypes.float8_e4m3fn)

    # identity (bf16) for the tail-layer PE transposes
    A["idnb"] = np.eye(128, dtype=f32).astype(ml_dtypes.bfloat16)

    # bias columns [128, 29]
    bc = np.zeros((H, 29), f32)
    for l in range(L):
        bc[:, 4 * l + 0] = np.asarray(inputs["bm1"], f32)[l]
        bc[:, 4 * l + 1] = np.asarray(inputs["bm2"], f32)[l]
        bc[:, 4 * l + 2] = np.asarray(inputs["be"], f32)[l]
        bc[:, 4 * l + 3] = np.asarray(inputs["bg"], f32)[l]
        bc[:, 12 + 2 * l + 0] = np.asarray(inputs["bo1"], f32)[l]
        bc[:, 12 + 2 * l + 1] = np.asarray(inputs["bo2"], f32)[l]
        bc[:, 18 + l] = np.asarray(inputs["bp1"], f32)[l]
        bc[:, 22 + l] = np.asarray(inputs["ln_s"], f32)[l]
        bc[:, 25 + l] = np.asarray(inputs["ln_b"], f32)[l]
    bc[:, 21] = EPS
    bc[:, 28] = np.asarray(inputs["bh1"], f32)
    A["bias_cols"] = bc
    A["bh2_full"] = np.ascontiguousarray(
        np.asarray(inputs["bh2"], f32).reshape(OUT, 1))
    return A


# --------------------------------------------------------------------------
# Bass program.
# --------------------------------------------------------------------------

def _build_program(struct, wmap, bmap, ws1, bs1, wcols, bcols2):
    import concourse.bacc as bacc
    import concourse.mybir as mybir
    import concourse.tile as tile

    F32 = mybir.dt.float32
    S_core = struct["S_core"]

    nc = bacc.Bacc("TRN2", target_bir_lowering=False, debug=False)

    FP8 = mybir.dt.float8e4
    F32R = mybir.dt.float32r
    d = {}
    d["d_gp"] = nc.dram_tensor("gp", [128, 2 * S_core], FP8, kind="ExternalInput")
    d["d_sohp"] = nc.dram_tensor("sohp", [48, 2 * S_core], FP8, kind="ExternalInput")
    d["d_xohp"] = nc.dram_tensor("xohp", [5, AV, 2 * BL * N], FP8, kind="ExternalInput")
    d["d_atomp"] = nc.dram_tensor("atomp", [AV, 30 * H], FP8, kind="ExternalInput")
    d["d_wblob"] = nc.dram_tensor("wblob", [128, wcols], F32R, kind="ExternalInput")
    d["d_wbb"] = nc.dram_tensor("wbb", [128, bcols2], mybir.dt.bfloat16,
                                kind="ExternalInput")
    d["d_idnb"] = nc.dram_tensor("idnb", [128, 128], mybir.dt.bfloat16,
                                 kind="ExternalInput")
    d["d_bc"] = nc.dram_tensor("bias_cols", [H, 29], F32, kind="ExternalInput")
    d["d_bh2"] = nc.dram_tensor("bh2_full", [OUT, 1], F32, kind="ExternalInput")
    if struct["has_empty"]:
        d["d_mask"] = nc.dram_tensor("maskrow", [1, BL * N], F32, kind="ExternalInput")
        d["d_neg"] = nc.dram_tensor("negrow", [1, BL * N], F32, kind="ExternalInput")
    d["d_out"] = nc.dram_tensor("out", [OUT, BL], F32, kind="ExternalOutput")

    with tile.TileContext(nc) as tc:
        _emit(tc, nc, d, struct, wmap, bmap, ws1, bs1, mybir)
    nc.compile()
    return nc


def _emit(tc, nc, d, struct, wmap, bmap, ws1, bs1, mybir):
    import contextlib
    ctx = contextlib.ExitStack()
    F32 = mybir.dt.float32
    F32R = mybir.dt.float32r
    BF16 = mybir.dt.bfloat16
    FP8 = mybir.dt.float8e4
    AF = mybir.ActivationFunctionType
    ALU = mybir.AluOpType
    AX = mybir.AxisListType
    DR = mybir.MatmulPerfMode.DoubleRow

    S_graph = struct["S_graph"]
    S_core = struct["S_core"]
    groups = struct["groups"]
    tile_used = struct["tile_used"]
    has_empty = struct["has_empty"]
    n_tiles = struct["n_tiles"]

    # ---- engine load balancer -------------------------------------------
    load = {"ACT": 0.0, "DVE": 0.0, "POOL": 0.0}
    OVH = {"ACT": 215.0, "DVE": 170.0, "POOL": 130.0}
    ENG = {"ACT": nc.scalar, "DVE": nc.vector, "POOL": nc.gpsimd}

    def rate(e, sbuf=False, b2=False):
        if e == "ACT":
            return 0.833
        if e == "POOL":
            return 1.389
        if sbuf and b2:
            return 0.26
        if sbuf:
            return 0.521
        return 1.042

    def pick(cands, cols, sbuf=False, b2=False):
        e = min(cands,
                key=lambda e: load[e] + cols * rate(e, sbuf, b2) + OVH[e])
        load[e] += cols * rate(e, sbuf, b2) + OVH[e]
        return e

    def charge(e, cols):
        load[e] += cols * rate(e) + OVH[e]

    def ew_relu(out, in_, bias_ap, cols, cands=("ACT", "DVE")):
        e = pick(cands, cols)
        if e == "ACT":
            nc.scalar.activation(out, in_, AF.Relu, bias=bias_ap)
        else:
            ENG[e].tensor_scalar(out, in_, bias_ap, 0.0,
                                 op0=ALU.add, op1=ALU.max)

    def ew_copy(out, in_, cols, cands=("ACT", "DVE"), scale=None):
        e = pick(cands, cols)
        if e == "ACT":
            if scale is None:
                nc.scalar.activation(out, in_, AF.Copy)
            else:
                nc.scalar.activation(out, in_, AF.Copy, scale=scale)
        elif scale is None:
            ENG[e].tensor_copy(out, in_)
        else:
            ENG[e].tensor_scalar(out, in_, scale, None, op0=ALU.mult)

    def ew_stt(out, in0, scalar, in1, op0, op1, cols, cands=("DVE", "POOL")):
        e = pick(cands, cols)
        ENG[e].scalar_tensor_tensor(out, in0, scalar, in1, op0=op0, op1=op1)

    def ew_ts2(out, in_, s1, s2, op0, op1, cols, cands=("DVE", "POOL"),
               sbuf=False, b2=False):
        e = pick(cands, cols, sbuf, b2)
        ENG[e].tensor_scalar(out, in_, s1, s2, op0=op0, op1=op1)

    def ew_scale_bias(out, in_, s_ap, b_ap, cols, cands=("ACT", "DVE", "POOL"),
                      sbuf=False, b2=False):
        e = pick(cands, cols, sbuf, b2)
        if e == "ACT":
            nc.scalar.activation(out, in_, AF.Identity, scale=s_ap, bias=b_ap)
        else:
            ENG[e].tensor_scalar(out, in_, s_ap, b_ap,
                                 op0=ALU.mult, op1=ALU.add)

    # ---- pools -----------------------------------------------------------
    pG = ctx.enter_context(tc.tile_pool(name="pG", bufs=1))
    pW = ctx.enter_context(tc.tile_pool(name="pW", bufs=1))
    pAct = ctx.enter_context(tc.tile_pool(name="pAct", bufs=4))
    pNM = ctx.enter_context(tc.tile_pool(name="pNM", bufs=1))
    pMB = ctx.enter_context(tc.tile_pool(name="pMB", bufs=2))
    pLN = ctx.enter_context(tc.tile_pool(name="pLN", bufs=2))
    ps_pre = ctx.enter_context(tc.tile_pool(name="ps_pre", bufs=2, space="PSUM"))
    ps_p1 = ctx.enter_context(tc.tile_pool(name="ps_p1", bufs=2, space="PSUM"))
    ps_p2 = ctx.enter_context(tc.tile_pool(name="ps_p2", bufs=2, space="PSUM"))
    ps_misc = ctx.enter_context(tc.tile_pool(name="ps_misc", bufs=2, space="PSUM"))

    def mps(name, dt=F32):
        return ps_misc.tile([128, 512], dt, name=name, tag="mps")

    # ---- ACT table preload + PE p-state warmup (during the DMA wait) -----
    dummy = pW.tile([1, 1], F32, name="dummy")
    nc.gpsimd.memset(dummy[:], 1.0)
    for fn in (AF.Relu, AF.Identity, AF.Sqrt, AF.Copy):
        nc.scalar.activation(dummy[:], dummy[:], fn)
    dumb = pW.tile([1, 1], BF16, name="dumb")
    nc.gpsimd.memset(dumb[:], 1.0)
    warm = ps_misc.tile([128, 512], F32, name="warm", tag="mps")
    for _ in range(12):
        nc.tensor.matmul(warm[0:1, 0:1], dumb[:], dumb[:],
                         start=True, stop=True)

    # ---- resident tiles + DMA schedule ----------------------------------
    atomp_sb = pW.tile([AV, 30 * H], FP8, name="atomp_sb")
    nc.sync.dma_start(atomp_sb[:], d["d_atomp"].ap())
    xohp_sb = pW.tile([AV, 10 * BL * N], FP8, name="xohp_sb")
    XW = 2 * BL * N
    nc.sync.dma_start(xohp_sb[:, 0:XW], d["d_xohp"].ap()[0])
    for p in range(1, 5):
        nc.sync.dma_start(xohp_sb[:, p * XW:(p + 1) * XW],
                          d["d_xohp"].ap()[p])
    gp_sb = pG.tile([128, 2 * S_core], FP8, name="gp_sb")
    sohp_sb = pG.tile([48, 2 * S_core], FP8, name="sohp_sb")
    SG2 = 2 * S_graph
    wcols = sum(w for (_, w) in wmap.values())
    wblob_sb = pW.tile([128, wcols], F32R, name="wblob_sb")
    nc.sync.dma_start(wblob_sb[:, 0:ws1], d["d_wblob"].ap()[:, 0:ws1])
    bcols2 = sum(w for (_, w) in bmap.values())
    wbb_sb = pW.tile([128, bcols2], BF16, name="wbb_sb")
    nc.sync.dma_start(wbb_sb[:, 0:bs1], d["d_wbb"].ap()[:, 0:bs1])
    nc.sync.dma_start(gp_sb[:, 0:SG2], d["d_gp"].ap()[:, 0:SG2])
    nc.sync.dma_start(sohp_sb[:, 0:SG2], d["d_sohp"].ap()[:, 0:SG2])
    bc_sb = pW.tile([H, 29], F32, name="bc_sb")
    nc.sync.dma_start(bc_sb[:], d["d_bc"].ap())
    for g in range(1, BL):
        sl = slice(g * SG2, (g + 1) * SG2)
        nc.sync.dma_start(gp_sb[:, sl], d["d_gp"].ap()[:, sl])
        nc.sync.dma_start(sohp_sb[:, sl], d["d_sohp"].ap()[:, sl])
    bh2_sb = pW.tile([OUT, 1], F32, name="bh2_sb")
    nc.sync.dma_start(bh2_sb[:], d["d_bh2"].ap())
    idnb_sb = pW.tile([128, 128], mybir.dt.bfloat16, name="idnb_sb")
    nc.sync.dma_start(idnb_sb[:], d["d_idnb"].ap())
    nc.sync.dma_start(wblob_sb[:, ws1:], d["d_wblob"].ap()[:, ws1:])
    nc.sync.dma_start(wbb_sb[:, bs1:], d["d_wbb"].ap()[:, bs1:])

    if has_empty:
        mrow_sb = pW.tile([1, BL * N], F32, name="mrow_sb")
        nc.sync.dma_start(mrow_sb[:], d["d_mask"].ap())
        nrow_sb = pW.tile([1, BL * N], F32, name="nrow_sb")
        nc.sync.dma_start(nrow_sb[:], d["d_neg"].ap())
        mask_bc = pW.tile([128, BL * N], F32, name="mask_bc")
        nc.gpsimd.partition_broadcast(mask_bc[:], mrow_sb[:])
        neg_bc = pW.tile([128, BL * N], F32, name="neg_bc")
        nc.gpsimd.partition_broadcast(neg_bc[:], nrow_sb[:])

    def W(name):
        off, w = wmap[name]
        return wblob_sb[:, off:off + w]

    def WB(name):
        off, w = bmap[name]
        return wbb_sb[:, off:off + w]

    def pair(ap):
        return ap.rearrange("p (two h) -> p two h", two=2)

    def mov_pair(ap):
        return ap.rearrange("p (w two) -> p two w", two=2)

    # ---- preamble compute (overlaps G DMAs) ------------------------------
    bias_pre = pW.tile([128, L], F32, name="bias_pre")
    nc.vector.tensor_reduce(
        bias_pre[:], bc_sb[:, 0:4 * L].rearrange("p (l f) -> p l f", l=L),
        axis=AX.X, op=ALU.add)
    bo12 = pW.tile([128, L], F32, name="bo12")
    nc.vector.tensor_reduce(
        bo12[:], bc_sb[:, 12:12 + 2 * L].rearrange("p (l f) -> p l f", l=L),
        axis=AX.X, op=ALU.add)

    # node features: 15 fp8 DoubleRow matmuls (5 pairs x 3 levels)
    nf_ps = mps("nf_ps")
    for p in range(5):
        xs = mov_pair(xohp_sb[:, p * XW:(p + 1) * XW])
        for v in range(3):
            blk = (p * 3 + v) * 2 * H
            nc.tensor.matmul(nf_ps[:, 0:BL * N],
                             pair(atomp_sb[:, blk:blk + 2 * H]), xs,
                             start=(p == 0 and v == 0),
                             stop=(p == 4 and v == 2), perf_mode=DR)
    nf = pNM.tile([128, BL * N], BF16, name="nf")
    nc.scalar.activation(nf[:], nf_ps[:, 0:BL * N], AF.Copy)

    def prep_layer(l):
        """bw (bond @ We, xQS basis) + bias_h for layer l; emitted right
        before the layer so chunk-B weight DMAs never block the PE stream."""
        bw_ps = mps("bw_ps")
        nc.tensor.matmul(bw_ps[0:48, 0:H], W("bondT"), W(f"We_{l}"),
                         start=True, stop=True)
        bwp = pW.tile([48, 2 * H], FP8, name=f"bwp{l}")
        nc.scalar.activation(bwp[:, 0:H], bw_ps[0:48, 0:H], AF.Copy, scale=QS)
        nc.vector.scalar_tensor_tensor(bwp[:, H:2 * H], bw_ps[0:48, 0:H], QS,
                                       bwp[:, 0:H], op0=ALU.mult,
                                       op1=ALU.subtract)
        bwp_l[l] = bwp

        bh_ps = mps("bh_ps")
        nc.tensor.matmul(bh_ps[:, 0:2], WB(f"Wo2_{l}"),
                         WB("bp2fb")[:, l:l + 2], start=True, stop=True)
        bias_h = pW.tile([128, 1], F32, name=f"bias_h{l}")
        nc.vector.tensor_tensor(bias_h[:], bh_ps[:, 0:1], bo12[:, l:l + 1],
                                op=ALU.add)
        bias_h_l[l] = bias_h

    bwp_l, bias_h_l = {}, {}
    prep_layer(0)

    # ---- layers (software-pipelined emission) ----------------------------
    ST = {}

    def ensure_state(l):
        if l in ST:
            return
        st = {}
        st["msgs_max"] = pLN.tile([128, BL * N], BF16, name=f"msgs_max{l}",
                                  tag="msgs_max", bufs=2)
        st["h_fm"] = pLN.tile([128, BL * N], BF16, name=f"h_fm{l}",
                              tag="h_fm", bufs=2)
        if l < L - 1:
            st["hid"] = pNM.tile([128, BL * N], BF16, name=f"hid{l}",
                                 tag=f"hid{l}")
        else:
            st["ge_sb"] = pLN.tile([128, BL], BF16, name="ge_sb", tag="ge_sb")
        ST[l] = st

    M12P = {}

    def do_m12(l, gg):
        ensure_state(l)
        gsl = slice(gg * N, (gg + 1) * N)
        ps_m = mps("ps_m")
        nc.tensor.matmul(ps_m[:, 0:2 * H], nf[:, gsl], WB(f"m12_{l}_0"),
                         start=True, stop=(l == 0))
        if l > 0:
            nc.tensor.matmul(ps_m[:, 0:2 * H], ST[l - 1]["hid"][:, gsl],
                             WB(f"m12_{l}_1"), start=False, stop=True)
        m12h = pMB.tile([128, 2 * H], FP8, name=f"m12h{gg}", tag=f"m12h{gg}")
        ew_copy(m12h[:], ps_m[:, 0:2 * H], 2 * H)
        M12P[(l, gg)] = pair(m12h[:])

    def do_tiles(l, gg, t0, t1):
        msgs_max = ST[l]["msgs_max"]
        m12h_p = M12P[(l, gg)]
        bwp_pair = pair(bwp_l[l][:])
        for t in range(t0, t1):
            w = tile_used[t]
            c0 = 2 * (gg * S_graph + t * 512)
            gps = mov_pair(gp_sb[:, c0:c0 + 2 * w])
            sps = mov_pair(sohp_sb[:, c0:c0 + 2 * w])
            pre = ps_pre.tile([128, 512], F32, name="pre")
            nc.tensor.matmul(pre[:, 0:w], m12h_p, gps,
                             start=True, stop=False, perf_mode=DR)
            nc.tensor.matmul(pre[:, 0:w], bwp_pair, sps,
                             start=False, stop=True, perf_mode=DR)
            msgs1 = pAct.tile([128, 512], F32R, name="msgs1", tag="msgs1")
            ew_relu(msgs1[:, 0:w], pre[:, 0:w], bias_pre[:, l:l + 1], w)
            p1 = ps_p1.tile([128, 512], F32, name="p1")
            nc.tensor.matmul(p1[:, 0:w], W(f"Wp1_{l}"), msgs1[:, 0:w],
                             start=True, stop=True)
            msgs2 = pAct.tile([128, 512], F32R, name="msgs2", tag="msgs2")
            ew_relu(msgs2[:, 0:w], p1[:, 0:w], bc_sb[:, 18 + l:19 + l], w)
            p2 = ps_p2.tile([128, 512], F32, name="p2")
            nc.tensor.matmul(p2[:, 0:w], W(f"Wp2_{l}"), msgs2[:, 0:w],
                             start=True, stop=True)
            for (p0, R, K, gt, off) in groups:
                if gt != t:
                    continue
                out_ap = msgs_max[:, gg * N + p0: gg * N + p0 + R]
                seg = p2[:, off:off + R * K].rearrange("p (r k) -> p r k", r=R)
                nc.vector.tensor_reduce(out_ap, seg, axis=AX.X, op=ALU.max)
                load["DVE"] += R * K * 1.042 + OVH["DVE"]

    def do_h(l, gg):
        msgs_max, h_fm = ST[l]["msgs_max"], ST[l]["h_fm"]
        gsl = slice(gg * N, (gg + 1) * N)
        if has_empty:
            mm = pLN.tile([128, N], F32, name="mmx", tag="mmx", bufs=4)
            nc.vector.tensor_tensor(mm[:], msgs_max[:, gsl],
                                    mask_bc[:, gsl], op=ALU.mult)
            nc.vector.tensor_tensor(msgs_max[:, gsl], mm[:],
                                    neg_bc[:, gsl], op=ALU.add)
        h_ps = mps("h_ps")
        nc.tensor.matmul(h_ps[:, 0:N], WB(f"Wo1_{l}_0"), nf[:, gsl],
                         start=True, stop=False)
        if l > 0:
            nc.tensor.matmul(h_ps[:, 0:N], WB(f"Wo1_{l}_1"),
                             ST[l - 1]["hid"][:, gsl], start=False, stop=False)
        nc.tensor.matmul(h_ps[:, 0:N], WB(f"Wo2_{l}"), msgs_max[:, gsl],
                         start=False, stop=True)
        ew_relu(h_fm[:, gsl], h_ps[:, 0:N], bias_h_l[l][:], N)

    def do_ln(l, gg):
        h_fm = ST[l]["h_fm"]
        gsl = slice(gg * N, (gg + 1) * N)
        if l < L - 1:
            # DMA-engine transpose: no PE stall, latency hidden by tiles
            hn = pLN.tile([128, 128], BF16, name="hn", tag="hn", bufs=4)
            nc.sync.dma_start_transpose(hn[:], h_fm[:, gsl])
        else:
            # tail: PE transpose (53ns) keeps the closing chain short
            tp = mps("tp_ps", BF16)
            nc.tensor.transpose(tp[:, 0:128], h_fm[:, gsl], idnb_sb[:])
            hn = tp[:, 0:128]
        st6 = pLN.tile([128, 6], F32, name="st6", tag="st6", bufs=4)
        nc.vector.bn_stats(st6[:], hn)
        charge("DVE", 128)
        st2 = pLN.tile([128, 2], F32, name="st2", tag="st2", bufs=4)
        nc.vector.bn_aggr(st2[:], st6[:])
        charge("DVE", 8)
        std = pLN.tile([128, 1], F32, name="std", tag="std", bufs=4)
        nc.scalar.activation(std[:], st2[:, 1:2], AF.Sqrt,
                             bias=bc_sb[:, 21:22])
        charge("ACT", 1)
        rstd = pLN.tile([128, 1], F32, name="rstd", tag="rstd", bufs=4)
        nc.vector.reciprocal(rstd[:], std[:])
        charge("DVE", 1)
        hnorm = pLN.tile([128, 128], BF16, name="hnorm", tag="hnorm", bufs=4)
        ln_cands = ("DVE", "POOL") if l < L - 1 else ("DVE",)
        ew_ts2(hnorm[:], hn, st2[:, 0:1], rstd[:],
               ALU.subtract, ALU.mult, 128, cands=ln_cands,
               sbuf=(l < L - 1), b2=(l < L - 1))
        if l < L - 1:
            hb = pLN.tile([128, 128], BF16, name="hb", tag="hb", bufs=4)
            nc.sync.dma_start_transpose(hb[:], hnorm[:])
            ew_scale_bias(ST[l]["hid"][:, gsl], hb[:],
                          bc_sb[:, 22 + l:23 + l], bc_sb[:, 25 + l:26 + l],
                          128, sbuf=True)
        else:
            ge_g = mps(f"ge_ps{gg}")
            nc.tensor.matmul(ge_g[:, 0:1], hnorm[:], WB("oneN"),
                             start=True, stop=True)
            nc.scalar.activation(ST[l]["ge_sb"][:, gg:gg + 1], ge_g[:, 0:1],
                                 AF.Identity, scale=bc_sb[:, 24:25],
                                 bias=bc_sb[:, 27:28])
            charge("ACT", 1)

    # pipelined schedule: per-graph h/LN chains are emitted mid-way through
    # the NEXT graph's tile stream (so the PE never waits on trailing
    # reduces), and layer l+1's first graphs overlap layer l's close.
    nsplit = min(3, n_tiles)
    for l in range(L):
        for i in range(BL):
            do_m12(l, i)
            do_tiles(l, i, 0, nsplit)
            if i == 0:
                if l > 0:
                    do_h(l - 1, 3)
                    do_ln(l - 1, 3)
                elif n_tiles > nsplit:
                    pass
            else:
                do_h(l, i - 1)
                do_ln(l, i - 1)
            if i == 2 and l < L - 1:
                prep_layer(l + 1)
            do_tiles(l, i, nsplit, n_tiles)
    do_h(L - 1, 3)
    do_ln(L - 1, 3)
    ge_sb = ST[L - 1]["ge_sb"]

    # ---- head ------------------------------------------------------------
    o1 = mps("o1_ps")
    nc.tensor.matmul(o1[:, 0:BL], WB("Wh1"), ge_sb[:], start=True, stop=True)
    t1 = pLN.tile([128, BL], BF16, name="t1", tag="t1")
    nc.scalar.activation(t1[:], o1[:, 0:BL], AF.Relu, bias=bc_sb[:, 28:29])
    o2 = mps("o2_ps")
    nc.tensor.matmul(o2[:, 0:BL], WB("Wh2"), t1[:], start=True, stop=True)
    out_sb = pLN.tile([OUT, BL], F32, name="out_sb", tag="out_sb")
    nc.scalar.activation(out_sb[:], o2[:, 0:BL], AF.Identity, bias=bh2_sb[:])
    nc.sync.dma_start(d["d_out"].ap(), out_sb[:])
    ctx.close()


# --------------------------------------------------------------------------
# Entry point.
# --------------------------------------------------------------------------

def build(inputs):
    struct, percore = _prep(inputs)
    A = _weight_arrays(inputs)
    wmap = A.pop("_wmap")
    bmap = A.pop("_bmap")
    ws1 = A.pop("_ws1")
    bs1 = A.pop("_bs1")
    key = (struct["S_graph"], struct["n_tiles"],
           tuple(struct["groups"]), struct["has_empty"])
    if key not in _CACHE:
        _CACHE[key] = _build_program(struct, wmap, bmap, ws1, bs1,
                                     A["wblob"].shape[1], A["wbb"].shape[1])
    nc = _CACHE[key]

    in_maps = []
    for c in range(M):
        im = dict(
            gp=percore["gp"][c], sohp=percore["sohp"][c],
            xohp=percore["xohp"][c],
        )
        if struct["has_empty"]:
            im["maskrow"] = percore["maskrow"][c:c + 1]
            im["negrow"] = percore["negrow"][c:c + 1]
        for k, v in A.items():
            im[k] = v
        in_maps.append(im)
    return nc, in_maps, struct


def kernel(**inputs):
    from concourse import bass_utils
    nc, in_maps, struct = build(inputs)
    res = bass_utils.run_bass_kernel_spmd(nc, in_maps, core_ids=list(range(M)))
    out = np.zeros((B, OUT), np.float32)
    for c in range(M):
        out[c * BL:(c + 1) * BL] = res.results[c]["out"].T
    return out


# revision 4
# speedup vs baseline: 1.0372x; 1.0050x over previous
"""Trainium2 Bass kernel for nn_BaselineModel_74509092651544 (CLRS-style MPNN).

Strategy (v2)
-------------
Data-parallel over graphs: 32 graphs -> 8 cores x 4 graphs.  Only the ~61k
unique (graph,src,dst) edge slots survive the masked max, so the message MLP
runs on a padded CSR slot layout.

v2 changes vs the 114us baseline:
  * All one-hot/gather matmuls use fp8 DoubleRow perf mode (0.5 cyc/col):
      - pre accumulation: planes (m1,m2 | Gsrc,Gdst) with hi/lo fp8 splits
        of m1/m2, bond term as (bw_hi,bw_lo | SOH,SOH).
      - node features: 15 DoubleRows over 3-level fp8 atom embeddings.
    Small-magnitude operands are quantized in a x256 basis (one-hot entries
    1/256, exactly representable) to dodge e4m3's subnormal floor.
  * h-matmuls batched per graph pair (ap=256 avoids the <256 f32r 4x penalty).
  * Elementwise work greedily load-balanced across ACT/DVE/GPSIMD(Pool);
    segmented max reduces (DVE-only) get an overlapping-halves "premax"
    (tensor_tensor max) on Pool when that lowers the peak engine load.
  * LayerNorm transposes run on the (idle) DMA engines via dma_start_transpose
    in bf16 -- no PE/PSUM involvement, and the LN elementwise ops become
    all-SBUF (DVE 2x/4x modes).  Stats via bn_stats/bn_aggr.  The final layer
    pools via per-graph hnorm^T @ (1/N) matmuls (no un-transpose).
  * bp2 folded into bias_h on-device (removes the per-graph msgs_used pass).
  * Compute-ordered DMA schedule; ACT tables preloaded during the DMA wait.
"""

import sys
import numpy as np

sys.path.insert(0, "/opt/trn_rl_repo")

B, N, H, L, E, OUT = 32, 128, 128, 3, 65536, 128
M = 8                 # NeuronCores
BL = B // M           # graphs per core
NEG = -1e9
EPS = 1e-5
AV, BV = 128, 16
QS = 256.0            # fp8 scale basis for small-magnitude operands

_CACHE = {}


def _f8split(arr, levels):
    """Split float array into `levels` fp8(e4m3) planes summing to ~arr."""
    import ml_dtypes
    FP8 = ml_dtypes.float8_e4m3fn
    out = []
    r = np.asarray(arr, np.float32)
    for _ in range(levels):
        q = r.astype(FP8)
        out.append(q)
        r = r - q.astype(np.float32)
    return out


# --------------------------------------------------------------------------
# Host preprocessing: integer indexing / relayout / dtype splits only.
# --------------------------------------------------------------------------

def _prep(inputs):
    import ml_dtypes
    FP8 = ml_dtypes.float8_e4m3fn
    x = np.asarray(inputs["x"]).astype(np.int64)            # [B*N, 9]
    ea = np.asarray(inputs["edge_attr"]).astype(np.int64)   # [E, 3]
    ei = np.asarray(inputs["edge_index"]).astype(np.int64)  # [2, E]

    g = ei[0] // N
    s = ei[0] % N
    d = ei[1] % N
    key = (g * N + s) * N + d
    uniq, inv = np.unique(key, return_inverse=True)
    US = uniq.size
    ug = uniq // (N * N)
    us = (uniq // N) % N
    ud = uniq % N

    # bond one-hot counts per unique slot  [US, 48]
    oh48 = np.zeros((US, 48), np.float32)
    for c in range(3):
        np.add.at(oh48, (inv, ea[:, c] + 16 * c), 1.0)

    # unique in-degree per (graph, receiver)
    deg = np.zeros((B, N), np.int64)
    np.add.at(deg, (ug, ud), 1)

    # receiver relabeling: position p holds the p-th highest-degree receiver
    rho = np.argsort(-deg, axis=1, kind="stable")        # [B, N] pos -> orig
    rho_inv = np.argsort(rho, axis=1)                    # orig -> pos
    degS = -np.sort(-deg, axis=1)                        # [B, N] desc
    Kp = np.maximum(degS.max(axis=0), 1)                 # [N]

    # group schedule (shared by all graphs/cores): (p0, R, K)
    groups = []
    p = 0
    while p < N:
        K = int(Kp[p])
        if 16 * K <= 512:
            R = 16
        elif 8 * K <= 512:
            R = 8
        else:
            R = 4
        R = min(R, N - p)
        groups.append((p, R, K))
        p += R

    def ffd(sizes):
        order_g = np.argsort(-np.asarray(sizes), kind="stable")
        tiles_used = []
        place = [None] * len(sizes)
        for gi in order_g:
            sz = sizes[gi]
            for t in range(len(tiles_used)):
                if tiles_used[t] + sz <= 512:
                    place[gi] = (t, tiles_used[t])
                    tiles_used[t] += sz
                    break
            else:
                place[gi] = (len(tiles_used), 0)
                tiles_used.append(sz)
        return place, tiles_used

    def cost(groups):
        place, tiles_used = ffd([R * K for (_, R, K) in groups])
        return (sum(tiles_used) * 1.042 + 170 * len(groups)
                + 2500 * len(tiles_used))

    # merge adjacent groups (padding the smaller K up) when it helps
    improved = True
    while improved:
        improved = False
        for i in range(len(groups) - 1):
            p0a, Ra, Ka = groups[i]
            p0b, Rb, Kb = groups[i + 1]
            if (Ra + Rb) * max(Ka, Kb) > 512:
                continue
            cand = (groups[:i] + [(p0a, Ra + Rb, max(Ka, Kb))]
                    + groups[i + 2:])
            if cost(cand) < cost(groups):
                groups = cand
                improved = True
                break

    place, tiles_used = ffd([R * K for (_, R, K) in groups])
    n_tiles = len(tiles_used)
    S_graph = 512 * n_tiles
    S_core = BL * S_graph

    # per-position lookup tables
    col_base_of_pos = np.zeros(N, np.int64)   # first column of the receiver
    K_of_pos = np.zeros(N, np.int64)
    for gi, (p0, R, K) in enumerate(groups):
        t, off = place[gi]
        for r in range(R):
            col_base_of_pos[p0 + r] = t * 512 + off + r * K
            K_of_pos[p0 + r] = K

    # slots ordered by (g, d, s): contiguous per receiver
    order = np.lexsort((us, ud, ug))
    og, od, osl = ug[order], ud[order], order
    osrc = us[order]
    recv_id = og * N + od
    first = np.concatenate([[0], np.flatnonzero(np.diff(recv_id)) + 1])
    k_rank = np.arange(len(og)) - first[np.searchsorted(recv_id[first], recv_id)]

    pos = rho_inv[og, od]
    core_r = og // BL
    col_r = (og % BL) * S_graph + col_base_of_pos[pos] + k_rank

    # padding: receivers with deg < K duplicate their first slot
    fg, fd = og[first], od[first]
    fpos = rho_inv[fg, fd]
    fdeg = deg[fg, fd]
    fK = K_of_pos[fpos]
    padc = (fK - fdeg).astype(np.int64)
    assert (padc >= 0).all()
    rep = np.repeat(np.arange(len(first)), padc)
    kpad = np.arange(len(rep)) - np.repeat(
        np.concatenate([[0], np.cumsum(padc)[:-1]]), padc
    ) + np.repeat(fdeg, padc)
    pg = fg[rep]
    core_p = pg // BL
    col_p = (pg % BL) * S_graph + col_base_of_pos[fpos[rep]] + kpad
    slot_p = osl[first][rep]
    src_p = osrc[first][rep]
    pos_p = fpos[rep]

    a_core = np.concatenate([core_r, core_p])
    a_col = np.concatenate([col_r, col_p])
    a_slot = np.concatenate([osl, slot_p])
    a_srcnew = np.concatenate([rho_inv[og, osrc], rho_inv[pg, src_p]])
    a_dstpos = np.concatenate([pos, pos_p])

    # pair-interleaved gather planes: col 2j = src one-hot, col 2j+1 = dst
    flat = a_core * S_core + a_col
    GP = np.zeros((M * S_core * 2, 128), np.float32)
    GP[2 * flat, a_srcnew] = 1.0
    GP[2 * flat + 1, a_dstpos] = 1.0
    # SOH in the 1/QS basis (counts/QS are exact in fp8); bw is scaled by QS
    SOHP = np.zeros((M * S_core * 2, 48), np.float32)
    SOHP[2 * flat] = oh48[a_slot] * (1.0 / QS)
    SOHP[2 * flat + 1] = oh48[a_slot] * (1.0 / QS)
    GP = np.ascontiguousarray(
        GP.reshape(M, 2 * S_core, 128).transpose(0, 2, 1)).astype(FP8)
    SOHP = np.ascontiguousarray(
        SOHP.reshape(M, 2 * S_core, 48).transpose(0, 2, 1)).astype(FP8)

    # atom one-hots (value 1/QS), feature-pair interleaved: [M, 5, AV, 2*BL*N]
    gg_ = np.repeat(np.arange(B), N)
    pp = np.tile(np.arange(N), B)
    orig = gg_ * N + rho[gg_, pp]                  # [B*N] column -> orig node
    BLN = BL * N
    XOHP = np.zeros((M, 5, AV, 2 * BLN), np.float32)
    mcol = np.tile(np.arange(BLN), M)
    mcore = np.repeat(np.arange(M), BLN)
    for c in range(9):
        XOHP[mcore, c // 2, x[orig, c], 2 * mcol + (c % 2)] = 1.0 / QS
    XOHP = np.ascontiguousarray(XOHP.transpose(0, 2, 1, 3).reshape(
        M, AV, 5 * 2 * BLN)).astype(FP8)

    # empty receivers (deg==0) -> need NEG mask path
    empt = (deg == 0)
    has_empty = bool(empt.any())
    maskrow = np.ones((M, BLN), np.float32)
    negrow = np.zeros((M, BLN), np.float32)
    if has_empty:
        eg, en = np.nonzero(empt)
        epos = rho_inv[eg, en]
        maskrow[eg // BL, (eg % BL) * N + epos] = 0.0
        negrow[eg // BL, (eg % BL) * N + epos] = NEG

    tile_used = [max(256, ((u + 7) // 8) * 8) for u in tiles_used]
    struct = dict(
        S_graph=S_graph, S_core=S_core, n_tiles=n_tiles,
        groups=[(p0, R, K, place[gi][0], place[gi][1])
                for gi, (p0, R, K) in enumerate(groups)],
        tile_used=tuple(tile_used),
        has_empty=has_empty,
    )
    percore = dict(gp=GP, sohp=SOHP, xohp=XOHP, maskrow=maskrow, negrow=negrow)
    return struct, percore


def _weight_arrays(inputs):
    f32 = np.float32
    A = {}

    import ml_dtypes
    Wm1 = np.asarray(inputs["Wm1"], f32)
    Wm2 = np.asarray(inputs["Wm2"], f32)
    atom = np.asarray(inputs["atom_emb"], f32)      # [9, AV, H]
    cols = []
    wmap = {}
    bcols = []
    bmap = {}

    def add(name, arr):
        wmap[name] = (sum(c.shape[1] for c in cols), arr.shape[1])
        cols.append(np.asarray(arr, f32))

    def addb(name, arr):
        bmap[name] = (sum(c.shape[1] for c in bcols), arr.shape[1])
        bcols.append(np.asarray(arr, f32))

    # f32r blob (operands of f32r matmuls); chunk A = preamble + layer 0
    bond_T = np.zeros((128, 48), f32)
    bond_T[:, :] = np.asarray(inputs["bond_emb"], f32).reshape(48, H).T
    add("bondT", bond_T)
    add("We_0", np.asarray(inputs["We"], f32)[0])
    add("Wp1_0", np.asarray(inputs["Wp1"], f32)[0])
    add("Wp2_0", np.asarray(inputs["Wp2"], f32)[0])
    ws1 = sum(c.shape[1] for c in cols)
    for l in range(1, L):
        add(f"We_{l}", np.asarray(inputs["We"], f32)[l])
        add(f"Wp1_{l}", np.asarray(inputs["Wp1"], f32)[l])
        add(f"Wp2_{l}", np.asarray(inputs["Wp2"], f32)[l])
    A["wblob"] = np.ascontiguousarray(np.concatenate(cols, 1))

    # bf16 blob (operands of bf16 matmuls); chunk A = layer 0
    addb("m12_0_0", np.concatenate([Wm1[0, 0:128], Wm2[0, 0:128]], 1))
    addb("Wo1_0_0", np.asarray(inputs["Wo1"], f32)[0, 0:128])
    addb("Wo2_0", np.asarray(inputs["Wo2"], f32)[0])
    bp2f = np.zeros((H, 4), f32)
    bp2f[:, :L] = np.asarray(inputs["bp2"], f32).T
    addb("bp2fb", bp2f)
    bs1 = sum(c.shape[1] for c in bcols)
    for l in range(1, L):
        addb(f"m12_{l}_0", np.concatenate([Wm1[l, 0:128], Wm2[l, 0:128]], 1))
        addb(f"m12_{l}_1", np.concatenate([Wm1[l, 128:256], Wm2[l, 128:256]], 1))
        addb(f"Wo1_{l}_0", np.asarray(inputs["Wo1"], f32)[l, 0:128])
        addb(f"Wo1_{l}_1", np.asarray(inputs["Wo1"], f32)[l, 128:256])
        addb(f"Wo2_{l}", np.asarray(inputs["Wo2"], f32)[l])
    addb("Wh1", np.asarray(inputs["Wh1"], f32))
    addb("Wh2", np.asarray(inputs["Wh2"], f32))
    addb("oneN", np.full((128, 1), 1.0 / N, f32))
    A["wbb"] = np.ascontiguousarray(
        np.concatenate(bcols, 1)).astype(ml_dtypes.bfloat16)
    A["_wmap"] = wmap
    A["_bmap"] = bmap
    A["_ws1"] = ws1
    A["_bs1"] = bs1

    # 3-level fp8 atom embeddings in the xQS basis, feature-pair plane layout
    at10 = np.zeros((10, AV, H), f32)
    at10[:9] = atom * QS
    blocks = []
    for p in range(5):
        lv0 = _f8split(at10[2 * p], 3)
        lv1 = _f8split(at10[2 * p + 1], 3)
        for v in range(3):
            blocks.append(np.concatenate(
                [lv0[v].astype(f32), lv1[v].astype(f32)], 1))  # [AV, 2H]
    import ml_dtypes
    A["atomp"] = np.ascontiguousarray(
        np.concatenate(blocks, 1)).astype(ml_dtypes.float8_e4m3fn)

    # identity (bf16) for the tail-layer PE transposes
    A["idnb"] = np.eye(128, dtype=f32).astype(ml_dtypes.bfloat16)

    # bias columns [128, 29]
    bc = np.zeros((H, 29), f32)
    for l in range(L):
        bc[:, 4 * l + 0] = np.asarray(inputs["bm1"], f32)[l]
        bc[:, 4 * l + 1] = np.asarray(inputs["bm2"], f32)[l]
        bc[:, 4 * l + 2] = np.asarray(inputs["be"], f32)[l]
        bc[:, 4 * l + 3] = np.asarray(inputs["bg"], f32)[l]
        bc[:, 12 + 2 * l + 0] = np.asarray(inputs["bo1"], f32)[l]
        bc[:, 12 + 2 * l + 1] = np.asarray(inputs["bo2"], f32)[l]
        bc[:, 18 + l] = np.asarray(inputs["bp1"], f32)[l]
        bc[:, 22 + l] = np.asarray(inputs["ln_s"], f32)[l]
        bc[:, 25 + l] = np.asarray(inputs["ln_b"], f32)[l]
    bc[:, 21] = EPS
    bc[:, 28] = np.asarray(inputs["bh1"], f32)
    A["bias_cols"] = bc
    A["bh2_full"] = np.ascontiguousarray(
        np.asarray(inputs["bh2"], f32).reshape(OUT, 1))
    return A


# --------------------------------------------------------------------------
# Bass program.
# --------------------------------------------------------------------------

def _build_program(struct, wmap, bmap, ws1, bs1, wcols, bcols2):
    import concourse.bacc as bacc
    import concourse.mybir as mybir
    import concourse.tile as tile

    F32 = mybir.dt.float32
    S_core = struct["S_core"]

    nc = bacc.Bacc("TRN2", target_bir_lowering=False, debug=False)

    FP8 = mybir.dt.float8e4
    F32R = mybir.dt.float32r
    d = {}
    d["d_gp"] = nc.dram_tensor("gp", [128, 2 * S_core], FP8, kind="ExternalInput")
    d["d_sohp"] = nc.dram_tensor("sohp", [48, 2 * S_core], FP8, kind="ExternalInput")
    d["d_xohp"] = nc.dram_tensor("xohp", [AV, 10 * BL * N], FP8, kind="ExternalInput")
    d["d_atomp"] = nc.dram_tensor("atomp", [AV, 30 * H], FP8, kind="ExternalInput")
    d["d_wblob"] = nc.dram_tensor("wblob", [128, wcols], F32R, kind="ExternalInput")
    d["d_wbb"] = nc.dram_tensor("wbb", [128, bcols2], mybir.dt.bfloat16,
                                kind="ExternalInput")
    d["d_idnb"] = nc.dram_tensor("idnb", [128, 128], mybir.dt.bfloat16,
                                 kind="ExternalInput")
    d["d_bc"] = nc.dram_tensor("bias_cols", [H, 29], F32, kind="ExternalInput")
    d["d_bh2"] = nc.dram_tensor("bh2_full", [OUT, 1], F32, kind="ExternalInput")
    if struct["has_empty"]:
        d["d_mask"] = nc.dram_tensor("maskrow", [1, BL * N], F32, kind="ExternalInput")
        d["d_neg"] = nc.dram_tensor("negrow", [1, BL * N], F32, kind="ExternalInput")
    d["d_out"] = nc.dram_tensor("out", [OUT, BL], F32, kind="ExternalOutput")

    with tile.TileContext(nc) as tc:
        _emit(tc, nc, d, struct, wmap, bmap, ws1, bs1, mybir)
    nc.compile()
    return nc


def _emit(tc, nc, d, struct, wmap, bmap, ws1, bs1, mybir):
    import contextlib
    ctx = contextlib.ExitStack()
    F32 = mybir.dt.float32
    F32R = mybir.dt.float32r
    BF16 = mybir.dt.bfloat16
    FP8 = mybir.dt.float8e4
    AF = mybir.ActivationFunctionType
    ALU = mybir.AluOpType
    AX = mybir.AxisListType
    DR = mybir.MatmulPerfMode.DoubleRow

    S_graph = struct["S_graph"]
    S_core = struct["S_core"]
    groups = struct["groups"]
    tile_used = struct["tile_used"]
    has_empty = struct["has_empty"]
    n_tiles = struct["n_tiles"]

    # ---- engine load balancer -------------------------------------------
    load = {"ACT": 0.0, "DVE": 0.0, "POOL": 0.0}
    OVH = {"ACT": 215.0, "DVE": 170.0, "POOL": 130.0}
    ENG = {"ACT": nc.scalar, "DVE": nc.vector, "POOL": nc.gpsimd}

    def rate(e, sbuf=False, b2=False):
        if e == "ACT":
            return 0.833
        if e == "POOL":
            return 1.389
        if sbuf and b2:
            return 0.26
        if sbuf:
            return 0.521
        return 1.042

    def pick(cands, cols, sbuf=False, b2=False):
        e = min(cands,
                key=lambda e: load[e] + cols * rate(e, sbuf, b2) + OVH[e])
        load[e] += cols * rate(e, sbuf, b2) + OVH[e]
        return e

    def charge(e, cols):
        load[e] += cols * rate(e) + OVH[e]

    def ew_relu(out, in_, bias_ap, cols, cands=("ACT", "DVE")):
        e = pick(cands, cols)
        if e == "ACT":
            nc.scalar.activation(out, in_, AF.Relu, bias=bias_ap)
        else:
            ENG[e].tensor_scalar(out, in_, bias_ap, 0.0,
                                 op0=ALU.add, op1=ALU.max)

    def ew_copy(out, in_, cols, cands=("ACT", "DVE"), scale=None):
        e = pick(cands, cols)
        if e == "ACT":
            if scale is None:
                nc.scalar.activation(out, in_, AF.Copy)
            else:
                nc.scalar.activation(out, in_, AF.Copy, scale=scale)
        elif scale is None:
            ENG[e].tensor_copy(out, in_)
        else:
            ENG[e].tensor_scalar(out, in_, scale, None, op0=ALU.mult)

    def ew_stt(out, in0, scalar, in1, op0, op1, cols, cands=("DVE", "POOL")):
        e = pick(cands, cols)
        ENG[e].scalar_tensor_tensor(out, in0, scalar, in1, op0=op0, op1=op1)

    def ew_ts2(out, in_, s1, s2, op0, op1, cols, cands=("DVE", "POOL"),
               sbuf=False, b2=False):
        e = pick(cands, cols, sbuf, b2)
        ENG[e].tensor_scalar(out, in_, s1, s2, op0=op0, op1=op1)

    def ew_scale_bias(out, in_, s_ap, b_ap, cols, cands=("ACT", "DVE", "POOL"),
                      sbuf=False, b2=False):
        e = pick(cands, cols, sbuf, b2)
        if e == "ACT":
            nc.scalar.activation(out, in_, AF.Identity, scale=s_ap, bias=b_ap)
        else:
            ENG[e].tensor_scalar(out, in_, s_ap, b_ap,
                                 op0=ALU.mult, op1=ALU.add)

    # ---- pools -----------------------------------------------------------
    pG = ctx.enter_context(tc.tile_pool(name="pG", bufs=1))
    pW = ctx.enter_context(tc.tile_pool(name="pW", bufs=1))
    pAct = ctx.enter_context(tc.tile_pool(name="pAct", bufs=6))
    pNM = ctx.enter_context(tc.tile_pool(name="pNM", bufs=1))
    pMB = ctx.enter_context(tc.tile_pool(name="pMB", bufs=2))
    pLN = ctx.enter_context(tc.tile_pool(name="pLN", bufs=2))
    ps_pre = ctx.enter_context(tc.tile_pool(name="ps_pre", bufs=2, space="PSUM"))
    ps_p1 = ctx.enter_context(tc.tile_pool(name="ps_p1", bufs=2, space="PSUM"))
    ps_p2 = ctx.enter_context(tc.tile_pool(name="ps_p2", bufs=2, space="PSUM"))
    ps_misc = ctx.enter_context(tc.tile_pool(name="ps_misc", bufs=2, space="PSUM"))

    def mps(name, dt=F32):
        return ps_misc.tile([128, 512], dt, name=name, tag="mps")

    # ---- ACT table preload + PE p-state warmup (during the DMA wait) -----
    dummy = pW.tile([1, 1], F32, name="dummy")
    nc.gpsimd.memset(dummy[:], 1.0)
    for fn in (AF.Relu, AF.Identity, AF.Sqrt, AF.Copy):
        nc.scalar.activation(dummy[:], dummy[:], fn)
    dumb = pW.tile([1, 1], BF16, name="dumb")
    nc.gpsimd.memset(dumb[:], 1.0)
    warm = ps_misc.tile([128, 512], F32, name="warm", tag="mps")
    for _ in range(12):
        nc.tensor.matmul(warm[0:1, 0:1], dumb[:], dumb[:],
                         start=True, stop=True)

    # ---- resident tiles + DMA schedule ----------------------------------
    atomp_sb = pW.tile([AV, 30 * H], FP8, name="atomp_sb")
    nc.sync.dma_start(atomp_sb[:], d["d_atomp"].ap())
    xohp_sb = pW.tile([AV, 10 * BL * N], FP8, name="xohp_sb")
    XW = 2 * BL * N
    nc.sync.dma_start(xohp_sb[:, 0:XW], d["d_xohp"].ap()[:, 0:XW])
    nc.sync.dma_start(xohp_sb[:, XW:5 * XW], d["d_xohp"].ap()[:, XW:5 * XW])
    gp_sb = pG.tile([128, 2 * S_core], FP8, name="gp_sb")
    sohp_sb = pG.tile([48, 2 * S_core], FP8, name="sohp_sb")
    SG2 = 2 * S_graph
    wcols = sum(w for (_, w) in wmap.values())
    wblob_sb = pW.tile([128, wcols], F32R, name="wblob_sb")
    nc.sync.dma_start(wblob_sb[:, 0:ws1], d["d_wblob"].ap()[:, 0:ws1])
    bcols2 = sum(w for (_, w) in bmap.values())
    wbb_sb = pW.tile([128, bcols2], BF16, name="wbb_sb")
    nc.sync.dma_start(wbb_sb[:, 0:bs1], d["d_wbb"].ap()[:, 0:bs1])
    nc.sync.dma_start(gp_sb[:, 0:SG2], d["d_gp"].ap()[:, 0:SG2])
    nc.sync.dma_start(sohp_sb[:, 0:SG2], d["d_sohp"].ap()[:, 0:SG2])
    bc_sb = pW.tile([H, 29], F32, name="bc_sb")
    nc.sync.dma_start(bc_sb[:], d["d_bc"].ap())
    for g in range(1, BL):
        sl = slice(g * SG2, (g + 1) * SG2)
        nc.sync.dma_start(gp_sb[:, sl], d["d_gp"].ap()[:, sl])
        nc.sync.dma_start(sohp_sb[:, sl], d["d_sohp"].ap()[:, sl])
    bh2_sb = pW.tile([OUT, 1], F32, name="bh2_sb")
    nc.sync.dma_start(bh2_sb[:], d["d_bh2"].ap())
    idnb_sb = pW.tile([128, 128], mybir.dt.bfloat16, name="idnb_sb")
    nc.sync.dma_start(idnb_sb[:], d["d_idnb"].ap())
    nc.sync.dma_start(wblob_sb[:, ws1:], d["d_wblob"].ap()[:, ws1:])
    nc.sync.dma_start(wbb_sb[:, bs1:], d["d_wbb"].ap()[:, bs1:])

    if has_empty:
        mrow_sb = pW.tile([1, BL * N], F32, name="mrow_sb")
        nc.sync.dma_start(mrow_sb[:], d["d_mask"].ap())
        nrow_sb = pW.tile([1, BL * N], F32, name="nrow_sb")
        nc.sync.dma_start(nrow_sb[:], d["d_neg"].ap())
        mask_bc = pW.tile([128, BL * N], F32, name="mask_bc")
        nc.gpsimd.partition_broadcast(mask_bc[:], mrow_sb[:])
        neg_bc = pW.tile([128, BL * N], F32, name="neg_bc")
        nc.gpsimd.partition_broadcast(neg_bc[:], nrow_sb[:])

    def W(name):
        off, w = wmap[name]
        return wblob_sb[:, off:off + w]

    def WB(name):
        off, w = bmap[name]
        return wbb_sb[:, off:off + w]

    def pair(ap):
        return ap.rearrange("p (two h) -> p two h", two=2)

    def mov_pair(ap):
        return ap.rearrange("p (w two) -> p two w", two=2)

    # ---- preamble compute (overlaps G DMAs) ------------------------------
    bias_pre = pW.tile([128, L], F32, name="bias_pre")
    nc.vector.tensor_reduce(
        bias_pre[:], bc_sb[:, 0:4 * L].rearrange("p (l f) -> p l f", l=L),
        axis=AX.X, op=ALU.add)
    bo12 = pW.tile([128, L], F32, name="bo12")
    nc.vector.tensor_reduce(
        bo12[:], bc_sb[:, 12:12 + 2 * L].rearrange("p (l f) -> p l f", l=L),
        axis=AX.X, op=ALU.add)

    # node features: 15 fp8 DoubleRow matmuls (5 pairs x 3 levels)
    nf_ps = mps("nf_ps")
    for p in range(5):
        xs = mov_pair(xohp_sb[:, p * XW:(p + 1) * XW])
        for v in range(3):
            blk = (p * 3 + v) * 2 * H
            nc.tensor.matmul(nf_ps[:, 0:BL * N],
                             pair(atomp_sb[:, blk:blk + 2 * H]), xs,
                             start=(p == 0 and v == 0),
                             stop=(p == 4 and v == 2), perf_mode=DR)
    nf = pNM.tile([128, BL * N], BF16, name="nf")
    nc.scalar.activation(nf[:], nf_ps[:, 0:BL * N], AF.Copy)

    def prep_layer(l):
        """bw (bond @ We, xQS basis) + bias_h for layer l; emitted right
        before the layer so chunk-B weight DMAs never block the PE stream."""
        bw_ps = mps("bw_ps")
        nc.tensor.matmul(bw_ps[0:48, 0:H], W("bondT"), W(f"We_{l}"),
                         start=True, stop=True)
        bwp = pW.tile([48, 2 * H], FP8, name=f"bwp{l}")
        nc.scalar.activation(bwp[:, 0:H], bw_ps[0:48, 0:H], AF.Copy, scale=QS)
        nc.vector.scalar_tensor_tensor(bwp[:, H:2 * H], bw_ps[0:48, 0:H], QS,
                                       bwp[:, 0:H], op0=ALU.mult,
                                       op1=ALU.subtract)
        bwp_l[l] = bwp

        bh_ps = mps("bh_ps")
        nc.tensor.matmul(bh_ps[:, 0:2], WB(f"Wo2_{l}"),
                         WB("bp2fb")[:, l:l + 2], start=True, stop=True)
        bias_h = pW.tile([128, 1], F32, name=f"bias_h{l}")
        nc.vector.tensor_tensor(bias_h[:], bh_ps[:, 0:1], bo12[:, l:l + 1],
                                op=ALU.add)
        bias_h_l[l] = bias_h

    bwp_l, bias_h_l = {}, {}
    prep_layer(0)

    # ---- layers (software-pipelined emission) ----------------------------
    ST = {}

    def ensure_state(l):
        if l in ST:
            return
        st = {}
        st["msgs_max"] = pLN.tile([128, BL * N], BF16, name=f"msgs_max{l}",
                                  tag="msgs_max", bufs=2)
        st["h_fm"] = pLN.tile([128, BL * N], BF16, name=f"h_fm{l}",
                              tag="h_fm", bufs=2)
        if l < L - 1:
            st["hid"] = pNM.tile([128, BL * N], BF16, name=f"hid{l}",
                                 tag=f"hid{l}")
        else:
            st["ge_sb"] = pLN.tile([128, BL], BF16, name="ge_sb", tag="ge_sb")
        ST[l] = st

    M12P = {}

    def do_m12(l, gg):
        ensure_state(l)
        gsl = slice(gg * N, (gg + 1) * N)
        ps_m = mps("ps_m")
        nc.tensor.matmul(ps_m[:, 0:2 * H], nf[:, gsl], WB(f"m12_{l}_0"),
                         start=True, stop=(l == 0))
        if l > 0:
            nc.tensor.matmul(ps_m[:, 0:2 * H], ST[l - 1]["hid"][:, gsl],
                             WB(f"m12_{l}_1"), start=False, stop=True)
        m12h = pMB.tile([128, 2 * H], FP8, name=f"m12h{gg}", tag=f"m12h{gg}")
        ew_copy(m12h[:], ps_m[:, 0:2 * H], 2 * H)
        M12P[(l, gg)] = pair(m12h[:])

    def do_tiles(l, gg, t0, t1):
        msgs_max = ST[l]["msgs_max"]
        m12h_p = M12P[(l, gg)]
        bwp_pair = pair(bwp_l[l][:])
        for t in range(t0, t1):
            w = tile_used[t]
            c0 = 2 * (gg * S_graph + t * 512)
            gps = mov_pair(gp_sb[:, c0:c0 + 2 * w])
            sps = mov_pair(sohp_sb[:, c0:c0 + 2 * w])
            pre = ps_pre.tile([128, 512], F32, name="pre")
            nc.tensor.matmul(pre[:, 0:w], m12h_p, gps,
                             start=True, stop=False, perf_mode=DR)
            nc.tensor.matmul(pre[:, 0:w], bwp_pair, sps,
                             start=False, stop=True, perf_mode=DR)
            msgs1 = pAct.tile([128, 512], F32R, name="msgs1", tag="msgs1")
            ew_relu(msgs1[:, 0:w], pre[:, 0:w], bias_pre[:, l:l + 1], w)
            p1 = ps_p1.tile([128, 512], F32, name="p1")
            nc.tensor.matmul(p1[:, 0:w], W(f"Wp1_{l}"), msgs1[:, 0:w],
                             start=True, stop=True)
            msgs2 = pAct.tile([128, 512], F32R, name="msgs2", tag="msgs2")
            ew_relu(msgs2[:, 0:w], p1[:, 0:w], bc_sb[:, 18 + l:19 + l], w)
            p2 = ps_p2.tile([128, 512], F32, name="p2")
            nc.tensor.matmul(p2[:, 0:w], W(f"Wp2_{l}"), msgs2[:, 0:w],
                             start=True, stop=True)
            for (p0, R, K, gt, off) in groups:
                if gt != t:
                    continue
                out_ap = msgs_max[:, gg * N + p0: gg * N + p0 + R]
                seg = p2[:, off:off + R * K].rearrange("p (r k) -> p r k", r=R)
                nc.vector.tensor_reduce(out_ap, seg, axis=AX.X, op=ALU.max)
                load["DVE"] += R * K * 1.042 + OVH["DVE"]

    def do_h(l, gg):
        msgs_max, h_fm = ST[l]["msgs_max"], ST[l]["h_fm"]
        gsl = slice(gg * N, (gg + 1) * N)
        if has_empty:
            mm = pLN.tile([128, N], F32, name="mmx", tag="mmx", bufs=4)
            nc.vector.tensor_tensor(mm[:], msgs_max[:, gsl],
                                    mask_bc[:, gsl], op=ALU.mult)
            nc.vector.tensor_tensor(msgs_max[:, gsl], mm[:],
                                    neg_bc[:, gsl], op=ALU.add)
        h_ps = mps("h_ps")
        nc.tensor.matmul(h_ps[:, 0:N], WB(f"Wo1_{l}_0"), nf[:, gsl],
                         start=True, stop=False)
        if l > 0:
            nc.tensor.matmul(h_ps[:, 0:N], WB(f"Wo1_{l}_1"),
                             ST[l - 1]["hid"][:, gsl], start=False, stop=False)
        nc.tensor.matmul(h_ps[:, 0:N], WB(f"Wo2_{l}"), msgs_max[:, gsl],
                         start=False, stop=True)
        ew_relu(h_fm[:, gsl], h_ps[:, 0:N], bias_h_l[l][:], N)

    def do_ln(l, gg):
        h_fm = ST[l]["h_fm"]
        gsl = slice(gg * N, (gg + 1) * N)
        if l < L - 1:
            # DMA-engine transpose: no PE stall, latency hidden by tiles
            hn = pLN.tile([128, 128], BF16, name="hn", tag="hn", bufs=4)
            nc.sync.dma_start_transpose(hn[:], h_fm[:, gsl])
        else:
            # tail: PE transpose (53ns) keeps the closing chain short
            tp = mps("tp_ps", BF16)
            nc.tensor.transpose(tp[:, 0:128], h_fm[:, gsl], idnb_sb[:])
            hn = tp[:, 0:128]
        st6 = pLN.tile([128, 6], F32, name="st6", tag="st6", bufs=4)
        nc.vector.bn_stats(st6[:], hn)
        charge("DVE", 128)
        st2 = pLN.tile([128, 2], F32, name="st2", tag="st2", bufs=4)
        nc.vector.bn_aggr(st2[:], st6[:])
        charge("DVE", 8)
        std = pLN.tile([128, 1], F32, name="std", tag="std", bufs=4)
        nc.scalar.activation(std[:], st2[:, 1:2], AF.Sqrt,
                             bias=bc_sb[:, 21:22])
        charge("ACT", 1)
        rstd = pLN.tile([128, 1], F32, name="rstd", tag="rstd", bufs=4)
        nc.vector.reciprocal(rstd[:], std[:])
        charge("DVE", 1)
        hnorm = pLN.tile([128, 128], BF16, name="hnorm", tag="hnorm", bufs=4)
        ln_cands = ("DVE", "POOL") if l < L - 1 else ("DVE",)
        ew_ts2(hnorm[:], hn, st2[:, 0:1], rstd[:],
               ALU.subtract, ALU.mult, 128, cands=ln_cands,
               sbuf=(l < L - 1), b2=(l < L - 1))
        if l < L - 1:
            hb = pLN.tile([128, 128], BF16, name="hb", tag="hb", bufs=4)
            nc.sync.dma_start_transpose(hb[:], hnorm[:])
            ew_scale_bias(ST[l]["hid"][:, gsl], hb[:],
                          bc_sb[:, 22 + l:23 + l], bc_sb[:, 25 + l:26 + l],
                          128, sbuf=True)
        else:
            ge_g = mps(f"ge_ps{gg}")
            nc.tensor.matmul(ge_g[:, 0:1], hnorm[:], WB("oneN"),
                             start=True, stop=True)
            nc.scalar.activation(ST[l]["ge_sb"][:, gg:gg + 1], ge_g[:, 0:1],
                                 AF.Identity, scale=bc_sb[:, 24:25],
                                 bias=bc_sb[:, 27:28])
            charge("ACT", 1)

    # pipelined schedule: per-graph h/LN chains are emitted mid-way through
    # the NEXT graph's tile stream (so the PE never waits on trailing
    # reduces), and layer l+1's first graphs overlap layer l's close.
    nsplit = min(3, n_tiles)
    for l in range(L):
        for i in range(BL):
            do_m12(l, i)
            do_tiles(l, i, 0, nsplit)
            if i == 0:
                if l > 0:
                    do_h(l - 1, 3)
                    do_ln(l - 1, 3)
                elif n_tiles > nsplit:
                    pass
            else:
                do_h(l, i - 1)
                do_ln(l, i - 1)
            if i == 2 and l < L - 1:
                prep_layer(l + 1)
            do_tiles(l, i, nsplit, n_tiles)
    do_h(L - 1, 3)
    do_ln(L - 1, 3)
    ge_sb = ST[L - 1]["ge_sb"]

    # ---- head ------------------------------------------------------------
    o1 = mps("o1_ps")
    nc.tensor.matmul(o1[:, 0:BL], WB("Wh1"), ge_sb[:], start=True, stop=True)
    t1 = pLN.tile([128, BL], BF16, name="t1", tag="t1")
    nc.scalar.activation(t1[:], o1[:, 0:BL], AF.Relu, bias=bc_sb[:, 28:29])
    o2 = mps("o2_ps")
    nc.tensor.matmul(o2[:, 0:BL], WB("Wh2"), t1[:], start=True, stop=True)
    out_sb = pLN.tile([OUT, BL], F32, name="out_sb", tag="out_sb")
    nc.scalar.activation(out_sb[:], o2[:, 0:BL], AF.Identity, bias=bh2_sb[:])
    nc.sync.dma_start(d["d_out"].ap(), out_sb[:])
    ctx.close()


# --------------------------------------------------------------------------
# Entry point.
# --------------------------------------------------------------------------

def build(inputs):
    struct, percore = _prep(inputs)
    A = _weight_arrays(inputs)
    wmap = A.pop("_wmap")
    bmap = A.pop("_bmap")
    ws1 = A.pop("_ws1")
    bs1 = A.pop("_bs1")
    key = (struct["S_graph"], struct["n_tiles"],
           tuple(struct["groups"]), struct["has_empty"])
    if key not in _CACHE:
        _CACHE[key] = _build_program(struct, wmap, bmap, ws1, bs1,
                                     A["wblob"].shape[1], A["wbb"].shape[1])
    nc = _CACHE[key]

    in_maps = []
    for c in range(M):
        im = dict(
            gp=percore["gp"][c], sohp=percore["sohp"][c],
            xohp=percore["xohp"][c],
        )
        if struct["has_empty"]:
            im["maskrow"] = percore["maskrow"][c:c + 1]
            im["negrow"] = percore["negrow"][c:c + 1]
        for k, v in A.items():
            im[k] = v
        in_maps.append(im)
    return nc, in_maps, struct


def kernel(**inputs):
    from concourse import bass_utils
    nc, in_maps, struct = build(inputs)
    res = bass_utils.run_bass_kernel_spmd(nc, in_maps, core_ids=list(range(M)))
    out = np.zeros((B, OUT), np.float32)
    for c in range(M):
        out[c * BL:(c + 1) * BL] = res.results[c]["out"].T
    return out


# revision 5
# speedup vs baseline: 1.0458x; 1.0082x over previous
"""Trainium2 Bass kernel for nn_BaselineModel_74509092651544 (CLRS-style MPNN).

Strategy (v2)
-------------
Data-parallel over graphs: 32 graphs -> 8 cores x 4 graphs.  Only the ~61k
unique (graph,src,dst) edge slots survive the masked max, so the message MLP
runs on a padded CSR slot layout.

v2 changes vs the 114us baseline:
  * All one-hot/gather matmuls use fp8 DoubleRow perf mode (0.5 cyc/col):
      - pre accumulation: planes (m1,m2 | Gsrc,Gdst) with hi/lo fp8 splits
        of m1/m2, bond term as (bw_hi,bw_lo | SOH,SOH).
      - node features: 15 DoubleRows over 3-level fp8 atom embeddings.
    Small-magnitude operands are quantized in a x256 basis (one-hot entries
    1/256, exactly representable) to dodge e4m3's subnormal floor.
  * h-matmuls batched per graph pair (ap=256 avoids the <256 f32r 4x penalty).
  * Elementwise work greedily load-balanced across ACT/DVE/GPSIMD(Pool);
    segmented max reduces (DVE-only) get an overlapping-halves "premax"
    (tensor_tensor max) on Pool when that lowers the peak engine load.
  * LayerNorm transposes run on the (idle) DMA engines via dma_start_transpose
    in bf16 -- no PE/PSUM involvement, and the LN elementwise ops become
    all-SBUF (DVE 2x/4x modes).  Stats via bn_stats/bn_aggr.  The final layer
    pools via per-graph hnorm^T @ (1/N) matmuls (no un-transpose).
  * bp2 folded into bias_h on-device (removes the per-graph msgs_used pass).
  * Compute-ordered DMA schedule; ACT tables preloaded during the DMA wait.
"""

import sys
import numpy as np

sys.path.insert(0, "/opt/trn_rl_repo")

B, N, H, L, E, OUT = 32, 128, 128, 3, 65536, 128
M = 8                 # NeuronCores
BL = B // M           # graphs per core
NEG = -1e9
EPS = 1e-5
AV, BV = 128, 16
QS = 256.0            # fp8 scale basis for small-magnitude operands

_CACHE = {}


def _f8split(arr, levels):
    """Split float array into `levels` fp8(e4m3) planes summing to ~arr."""
    import ml_dtypes
    FP8 = ml_dtypes.float8_e4m3fn
    out = []
    r = np.asarray(arr, np.float32)
    for _ in range(levels):
        q = r.astype(FP8)
        out.append(q)
        r = r - q.astype(np.float32)
    return out


# --------------------------------------------------------------------------
# Host preprocessing: integer indexing / relayout / dtype splits only.
# --------------------------------------------------------------------------

def _prep(inputs):
    import ml_dtypes
    FP8 = ml_dtypes.float8_e4m3fn
    x = np.asarray(inputs["x"]).astype(np.int64)            # [B*N, 9]
    ea = np.asarray(inputs["edge_attr"]).astype(np.int64)   # [E, 3]
    ei = np.asarray(inputs["edge_index"]).astype(np.int64)  # [2, E]

    g = ei[0] // N
    s = ei[0] % N
    d = ei[1] % N
    key = (g * N + s) * N + d
    uniq, inv = np.unique(key, return_inverse=True)
    US = uniq.size
    ug = uniq // (N * N)
    us = (uniq // N) % N
    ud = uniq % N

    # bond one-hot counts per unique slot  [US, 48]
    oh48 = np.zeros((US, 48), np.float32)
    for c in range(3):
        np.add.at(oh48, (inv, ea[:, c] + 16 * c), 1.0)

    # unique in-degree per (graph, receiver)
    deg = np.zeros((B, N), np.int64)
    np.add.at(deg, (ug, ud), 1)

    # receiver relabeling: position p holds the p-th highest-degree receiver
    rho = np.argsort(-deg, axis=1, kind="stable")        # [B, N] pos -> orig
    rho_inv = np.argsort(rho, axis=1)                    # orig -> pos
    degS = -np.sort(-deg, axis=1)                        # [B, N] desc
    Kp = np.maximum(degS.max(axis=0), 1)                 # [N]

    # group schedule (shared by all graphs/cores): (p0, R, K)
    groups = []
    p = 0
    while p < N:
        K = int(Kp[p])
        if 16 * K <= 512:
            R = 16
        elif 8 * K <= 512:
            R = 8
        else:
            R = 4
        R = min(R, N - p)
        groups.append((p, R, K))
        p += R

    def ffd(sizes):
        order_g = np.argsort(-np.asarray(sizes), kind="stable")
        tiles_used = []
        place = [None] * len(sizes)
        for gi in order_g:
            sz = sizes[gi]
            for t in range(len(tiles_used)):
                if tiles_used[t] + sz <= 512:
                    place[gi] = (t, tiles_used[t])
                    tiles_used[t] += sz
                    break
            else:
                place[gi] = (len(tiles_used), 0)
                tiles_used.append(sz)
        return place, tiles_used

    def cost(groups):
        place, tiles_used = ffd([R * K for (_, R, K) in groups])
        return (sum(tiles_used) * 1.042 + 170 * len(groups)
                + 2500 * len(tiles_used))

    # merge adjacent groups (padding the smaller K up) when it helps
    improved = True
    while improved:
        improved = False
        for i in range(len(groups) - 1):
            p0a, Ra, Ka = groups[i]
            p0b, Rb, Kb = groups[i + 1]
            if (Ra + Rb) * max(Ka, Kb) > 512:
                continue
            cand = (groups[:i] + [(p0a, Ra + Rb, max(Ka, Kb))]
                    + groups[i + 2:])
            if cost(cand) < cost(groups):
                groups = cand
                improved = True
                break

    place, tiles_used = ffd([R * K for (_, R, K) in groups])
    n_tiles = len(tiles_used)
    S_graph = 512 * n_tiles
    S_core = BL * S_graph

    # per-position lookup tables
    col_base_of_pos = np.zeros(N, np.int64)   # first column of the receiver
    K_of_pos = np.zeros(N, np.int64)
    for gi, (p0, R, K) in enumerate(groups):
        t, off = place[gi]
        for r in range(R):
            col_base_of_pos[p0 + r] = t * 512 + off + r * K
            K_of_pos[p0 + r] = K

    # slots ordered by (g, d, s): contiguous per receiver
    order = np.lexsort((us, ud, ug))
    og, od, osl = ug[order], ud[order], order
    osrc = us[order]
    recv_id = og * N + od
    first = np.concatenate([[0], np.flatnonzero(np.diff(recv_id)) + 1])
    k_rank = np.arange(len(og)) - first[np.searchsorted(recv_id[first], recv_id)]

    pos = rho_inv[og, od]
    core_r = og // BL
    col_r = (og % BL) * S_graph + col_base_of_pos[pos] + k_rank

    # padding: receivers with deg < K duplicate their first slot
    fg, fd = og[first], od[first]
    fpos = rho_inv[fg, fd]
    fdeg = deg[fg, fd]
    fK = K_of_pos[fpos]
    padc = (fK - fdeg).astype(np.int64)
    assert (padc >= 0).all()
    rep = np.repeat(np.arange(len(first)), padc)
    kpad = np.arange(len(rep)) - np.repeat(
        np.concatenate([[0], np.cumsum(padc)[:-1]]), padc
    ) + np.repeat(fdeg, padc)
    pg = fg[rep]
    core_p = pg // BL
    col_p = (pg % BL) * S_graph + col_base_of_pos[fpos[rep]] + kpad
    slot_p = osl[first][rep]
    src_p = osrc[first][rep]
    pos_p = fpos[rep]

    a_core = np.concatenate([core_r, core_p])
    a_col = np.concatenate([col_r, col_p])
    a_slot = np.concatenate([osl, slot_p])
    a_srcnew = np.concatenate([rho_inv[og, osrc], rho_inv[pg, src_p]])
    a_dstpos = np.concatenate([pos, pos_p])

    # pair-interleaved gather planes: col 2j = src one-hot, col 2j+1 = dst
    flat = a_core * S_core + a_col
    GP = np.zeros((M * S_core * 2, 128), np.float32)
    GP[2 * flat, a_srcnew] = 1.0
    GP[2 * flat + 1, a_dstpos] = 1.0
    # SOH in the 1/QS basis (counts/QS are exact in fp8); bw is scaled by QS
    SOHP = np.zeros((M * S_core * 2, 48), np.float32)
    SOHP[2 * flat] = oh48[a_slot] * (1.0 / QS)
    SOHP[2 * flat + 1] = oh48[a_slot] * (1.0 / QS)
    GP = np.ascontiguousarray(
        GP.reshape(M, 2 * S_core, 128).transpose(0, 2, 1)).astype(FP8)
    SOHP = np.ascontiguousarray(
        SOHP.reshape(M, 2 * S_core, 48).transpose(0, 2, 1)).astype(FP8)

    # atom one-hots (value 1/QS), feature-pair interleaved: [M, 5, AV, 2*BL*N]
    gg_ = np.repeat(np.arange(B), N)
    pp = np.tile(np.arange(N), B)
    orig = gg_ * N + rho[gg_, pp]                  # [B*N] column -> orig node
    BLN = BL * N
    XOHP = np.zeros((M, 5, AV, 2 * BLN), np.float32)
    mcol = np.tile(np.arange(BLN), M)
    mcore = np.repeat(np.arange(M), BLN)
    for c in range(9):
        XOHP[mcore, c // 2, x[orig, c], 2 * mcol + (c % 2)] = 1.0 / QS
    XOHP = np.ascontiguousarray(XOHP.transpose(0, 2, 1, 3).reshape(
        M, AV, 5 * 2 * BLN)).astype(FP8)

    # empty receivers (deg==0) -> need NEG mask path
    empt = (deg == 0)
    has_empty = bool(empt.any())
    maskrow = np.ones((M, BLN), np.float32)
    negrow = np.zeros((M, BLN), np.float32)
    if has_empty:
        eg, en = np.nonzero(empt)
        epos = rho_inv[eg, en]
        maskrow[eg // BL, (eg % BL) * N + epos] = 0.0
        negrow[eg // BL, (eg % BL) * N + epos] = NEG

    tile_used = [max(256, ((u + 7) // 8) * 8) for u in tiles_used]
    struct = dict(
        S_graph=S_graph, S_core=S_core, n_tiles=n_tiles,
        groups=[(p0, R, K, place[gi][0], place[gi][1])
                for gi, (p0, R, K) in enumerate(groups)],
        tile_used=tuple(tile_used),
        has_empty=has_empty,
    )
    percore = dict(gp=GP, sohp=SOHP, xohp=XOHP, maskrow=maskrow, negrow=negrow)
    return struct, percore


def _weight_arrays(inputs):
    f32 = np.float32
    A = {}

    import ml_dtypes
    Wm1 = np.asarray(inputs["Wm1"], f32)
    Wm2 = np.asarray(inputs["Wm2"], f32)
    atom = np.asarray(inputs["atom_emb"], f32)      # [9, AV, H]
    cols = []
    wmap = {}
    bcols = []
    bmap = {}

    def add(name, arr):
        wmap[name] = (sum(c.shape[1] for c in cols), arr.shape[1])
        cols.append(np.asarray(arr, f32))

    def addb(name, arr):
        bmap[name] = (sum(c.shape[1] for c in bcols), arr.shape[1])
        bcols.append(np.asarray(arr, f32))

    # f32r blob (operands of f32r matmuls); chunk A = preamble + layer 0
    bond_T = np.zeros((128, 48), f32)
    bond_T[:, :] = np.asarray(inputs["bond_emb"], f32).reshape(48, H).T
    add("bondT", bond_T)
    add("We_0", np.asarray(inputs["We"], f32)[0])
    add("Wp1_0", np.asarray(inputs["Wp1"], f32)[0])
    add("Wp2_0", np.asarray(inputs["Wp2"], f32)[0])
    ws1 = sum(c.shape[1] for c in cols)
    for l in range(1, L):
        add(f"We_{l}", np.asarray(inputs["We"], f32)[l])
        add(f"Wp1_{l}", np.asarray(inputs["Wp1"], f32)[l])
        add(f"Wp2_{l}", np.asarray(inputs["Wp2"], f32)[l])
    A["wblob"] = np.ascontiguousarray(np.concatenate(cols, 1))

    # bf16 blob (operands of bf16 matmuls); chunk A = layer 0
    addb("m12_0_0", np.concatenate([Wm1[0, 0:128], Wm2[0, 0:128]], 1))
    addb("Wo1_0_0", np.asarray(inputs["Wo1"], f32)[0, 0:128])
    addb("Wo2_0", np.asarray(inputs["Wo2"], f32)[0])
    bp2f = np.zeros((H, 4), f32)
    bp2f[:, :L] = np.asarray(inputs["bp2"], f32).T
    addb("bp2fb", bp2f)
    bs1 = sum(c.shape[1] for c in bcols)
    for l in range(1, L):
        addb(f"m12_{l}_0", np.concatenate([Wm1[l, 0:128], Wm2[l, 0:128]], 1))
        addb(f"m12_{l}_1", np.concatenate([Wm1[l, 128:256], Wm2[l, 128:256]], 1))
        addb(f"Wo1_{l}_0", np.asarray(inputs["Wo1"], f32)[l, 0:128])
        addb(f"Wo1_{l}_1", np.asarray(inputs["Wo1"], f32)[l, 128:256])
        addb(f"Wo2_{l}", np.asarray(inputs["Wo2"], f32)[l])
    addb("Wh1", np.asarray(inputs["Wh1"], f32))
    addb("Wh2", np.asarray(inputs["Wh2"], f32))
    addb("oneN", np.full((128, 1), 1.0 / N, f32))
    A["wbb"] = np.ascontiguousarray(
        np.concatenate(bcols, 1)).astype(ml_dtypes.bfloat16)
    A["_wmap"] = wmap
    A["_bmap"] = bmap
    A["_ws1"] = ws1
    A["_bs1"] = bs1

    # 3-level fp8 atom embeddings in the xQS basis, feature-pair plane layout
    at10 = np.zeros((10, AV, H), f32)
    at10[:9] = atom * QS
    blocks = []
    for p in range(5):
        lv0 = _f8split(at10[2 * p], 3)
        lv1 = _f8split(at10[2 * p + 1], 3)
        for v in range(3):
            blocks.append(np.concatenate(
                [lv0[v].astype(f32), lv1[v].astype(f32)], 1))  # [AV, 2H]
    import ml_dtypes
    A["atomp"] = np.ascontiguousarray(
        np.concatenate(blocks, 1)).astype(ml_dtypes.float8_e4m3fn)

    # identity (bf16) for the tail-layer PE transposes
    A["idnb"] = np.eye(128, dtype=f32).astype(ml_dtypes.bfloat16)

    # bias columns [128, 29]
    bc = np.zeros((H, 29), f32)
    for l in range(L):
        bc[:, 4 * l + 0] = np.asarray(inputs["bm1"], f32)[l]
        bc[:, 4 * l + 1] = np.asarray(inputs["bm2"], f32)[l]
        bc[:, 4 * l + 2] = np.asarray(inputs["be"], f32)[l]
        bc[:, 4 * l + 3] = np.asarray(inputs["bg"], f32)[l]
        bc[:, 12 + 2 * l + 0] = np.asarray(inputs["bo1"], f32)[l]
        bc[:, 12 + 2 * l + 1] = np.asarray(inputs["bo2"], f32)[l]
        bc[:, 18 + l] = np.asarray(inputs["bp1"], f32)[l]
        bc[:, 22 + l] = np.asarray(inputs["ln_s"], f32)[l]
        bc[:, 25 + l] = np.asarray(inputs["ln_b"], f32)[l]
    bc[:, 21] = EPS
    bc[:, 28] = np.asarray(inputs["bh1"], f32)
    A["bias_cols"] = bc
    A["bh2_full"] = np.ascontiguousarray(
        np.asarray(inputs["bh2"], f32).reshape(OUT, 1))
    return A


# --------------------------------------------------------------------------
# Bass program.
# --------------------------------------------------------------------------

def _build_program(struct, wmap, bmap, ws1, bs1, wcols, bcols2):
    import concourse.bacc as bacc
    import concourse.mybir as mybir
    import concourse.tile as tile

    F32 = mybir.dt.float32
    S_core = struct["S_core"]

    nc = bacc.Bacc("TRN2", target_bir_lowering=False, debug=False)

    FP8 = mybir.dt.float8e4
    F32R = mybir.dt.float32r
    d = {}
    d["d_gp"] = nc.dram_tensor("gp", [128, 2 * S_core], FP8, kind="ExternalInput")
    d["d_sohp"] = nc.dram_tensor("sohp", [48, 2 * S_core], FP8, kind="ExternalInput")
    d["d_xohp"] = nc.dram_tensor("xohp", [AV, 10 * BL * N], FP8, kind="ExternalInput")
    d["d_atomp"] = nc.dram_tensor("atomp", [AV, 30 * H], FP8, kind="ExternalInput")
    d["d_wblob"] = nc.dram_tensor("wblob", [128, wcols], F32R, kind="ExternalInput")
    d["d_wbb"] = nc.dram_tensor("wbb", [128, bcols2], mybir.dt.bfloat16,
                                kind="ExternalInput")
    d["d_idnb"] = nc.dram_tensor("idnb", [128, 128], mybir.dt.bfloat16,
                                 kind="ExternalInput")
    d["d_bc"] = nc.dram_tensor("bias_cols", [H, 29], F32, kind="ExternalInput")
    d["d_bh2"] = nc.dram_tensor("bh2_full", [OUT, 1], F32, kind="ExternalInput")
    if struct["has_empty"]:
        d["d_mask"] = nc.dram_tensor("maskrow", [1, BL * N], F32, kind="ExternalInput")
        d["d_neg"] = nc.dram_tensor("negrow", [1, BL * N], F32, kind="ExternalInput")
    d["d_out"] = nc.dram_tensor("out", [OUT, BL], F32, kind="ExternalOutput")

    with tile.TileContext(nc) as tc:
        _emit(tc, nc, d, struct, wmap, bmap, ws1, bs1, mybir)
    nc.compile()
    return nc


def _emit(tc, nc, d, struct, wmap, bmap, ws1, bs1, mybir):
    import contextlib
    ctx = contextlib.ExitStack()
    F32 = mybir.dt.float32
    F32R = mybir.dt.float32r
    BF16 = mybir.dt.bfloat16
    FP8 = mybir.dt.float8e4
    AF = mybir.ActivationFunctionType
    ALU = mybir.AluOpType
    AX = mybir.AxisListType
    DR = mybir.MatmulPerfMode.DoubleRow

    S_graph = struct["S_graph"]
    S_core = struct["S_core"]
    groups = struct["groups"]
    tile_used = struct["tile_used"]
    has_empty = struct["has_empty"]
    n_tiles = struct["n_tiles"]

    # ---- engine load balancer -------------------------------------------
    load = {"ACT": 0.0, "DVE": 0.0, "POOL": 0.0}
    OVH = {"ACT": 215.0, "DVE": 170.0, "POOL": 130.0}
    ENG = {"ACT": nc.scalar, "DVE": nc.vector, "POOL": nc.gpsimd}

    def rate(e, sbuf=False, b2=False):
        if e == "ACT":
            return 0.833
        if e == "POOL":
            return 1.389
        if sbuf and b2:
            return 0.26
        if sbuf:
            return 0.521
        return 1.042

    def pick(cands, cols, sbuf=False, b2=False):
        e = min(cands,
                key=lambda e: load[e] + cols * rate(e, sbuf, b2) + OVH[e])
        load[e] += cols * rate(e, sbuf, b2) + OVH[e]
        return e

    def charge(e, cols):
        load[e] += cols * rate(e) + OVH[e]

    def ew_relu(out, in_, bias_ap, cols, cands=("ACT", "DVE")):
        e = pick(cands, cols)
        if e == "ACT":
            nc.scalar.activation(out, in_, AF.Relu, bias=bias_ap)
        else:
            ENG[e].tensor_scalar(out, in_, bias_ap, 0.0,
                                 op0=ALU.add, op1=ALU.max)

    def ew_copy(out, in_, cols, cands=("ACT", "DVE"), scale=None):
        e = pick(cands, cols)
        if e == "ACT":
            if scale is None:
                nc.scalar.activation(out, in_, AF.Copy)
            else:
                nc.scalar.activation(out, in_, AF.Copy, scale=scale)
        elif scale is None:
            ENG[e].tensor_copy(out, in_)
        else:
            ENG[e].tensor_scalar(out, in_, scale, None, op0=ALU.mult)

    def ew_stt(out, in0, scalar, in1, op0, op1, cols, cands=("DVE", "POOL")):
        e = pick(cands, cols)
        ENG[e].scalar_tensor_tensor(out, in0, scalar, in1, op0=op0, op1=op1)

    def ew_ts2(out, in_, s1, s2, op0, op1, cols, cands=("DVE", "POOL"),
               sbuf=False, b2=False):
        e = pick(cands, cols, sbuf, b2)
        ENG[e].tensor_scalar(out, in_, s1, s2, op0=op0, op1=op1)

    def ew_scale_bias(out, in_, s_ap, b_ap, cols, cands=("ACT", "DVE", "POOL"),
                      sbuf=False, b2=False):
        e = pick(cands, cols, sbuf, b2)
        if e == "ACT":
            nc.scalar.activation(out, in_, AF.Identity, scale=s_ap, bias=b_ap)
        else:
            ENG[e].tensor_scalar(out, in_, s_ap, b_ap,
                                 op0=ALU.mult, op1=ALU.add)

    # ---- pools -----------------------------------------------------------
    pG = ctx.enter_context(tc.tile_pool(name="pG", bufs=1))
    pW = ctx.enter_context(tc.tile_pool(name="pW", bufs=1))
    pAct = ctx.enter_context(tc.tile_pool(name="pAct", bufs=6))
    pNM = ctx.enter_context(tc.tile_pool(name="pNM", bufs=1))
    pMB = ctx.enter_context(tc.tile_pool(name="pMB", bufs=2))
    pLN = ctx.enter_context(tc.tile_pool(name="pLN", bufs=2))
    ps_pre = ctx.enter_context(tc.tile_pool(name="ps_pre", bufs=2, space="PSUM"))
    ps_p1 = ctx.enter_context(tc.tile_pool(name="ps_p1", bufs=2, space="PSUM"))
    ps_p2 = ctx.enter_context(tc.tile_pool(name="ps_p2", bufs=2, space="PSUM"))
    ps_misc = ctx.enter_context(tc.tile_pool(name="ps_misc", bufs=2, space="PSUM"))

    def mps(name, dt=F32):
        return ps_misc.tile([128, 512], dt, name=name, tag="mps")

    # ---- ACT table preload + PE p-state warmup (during the DMA wait) -----
    dummy = pW.tile([1, 1], F32, name="dummy")
    nc.gpsimd.memset(dummy[:], 1.0)
    for fn in (AF.Relu, AF.Identity, AF.Sqrt, AF.Copy):
        nc.scalar.activation(dummy[:], dummy[:], fn)
    dumb = pW.tile([1, 1], BF16, name="dumb")
    nc.gpsimd.memset(dumb[:], 1.0)
    warm = ps_misc.tile([128, 512], F32, name="warm", tag="mps")
    for _ in range(12):
        nc.tensor.matmul(warm[0:1, 0:1], dumb[:], dumb[:],
                         start=True, stop=True)

    # ---- resident tiles + DMA schedule ----------------------------------
    atomp_sb = pW.tile([AV, 30 * H], FP8, name="atomp_sb")
    nc.sync.dma_start(atomp_sb[:], d["d_atomp"].ap())
    xohp_sb = pW.tile([AV, 10 * BL * N], FP8, name="xohp_sb")
    XW = 2 * BL * N
    nc.sync.dma_start(xohp_sb[:, 0:XW], d["d_xohp"].ap()[:, 0:XW])
    nc.sync.dma_start(xohp_sb[:, XW:5 * XW], d["d_xohp"].ap()[:, XW:5 * XW])
    gp_sb = pG.tile([128, 2 * S_core], FP8, name="gp_sb")
    sohp_sb = pG.tile([48, 2 * S_core], FP8, name="sohp_sb")
    SG2 = 2 * S_graph
    wcols = sum(w for (_, w) in wmap.values())
    wblob_sb = pW.tile([128, wcols], F32R, name="wblob_sb")
    nc.sync.dma_start(wblob_sb[:, 0:ws1], d["d_wblob"].ap()[:, 0:ws1])
    bcols2 = sum(w for (_, w) in bmap.values())
    wbb_sb = pW.tile([128, bcols2], BF16, name="wbb_sb")
    nc.sync.dma_start(wbb_sb[:, 0:bs1], d["d_wbb"].ap()[:, 0:bs1])
    nc.sync.dma_start(gp_sb[:, 0:2048], d["d_gp"].ap()[:, 0:2048])
    nc.sync.dma_start(sohp_sb[:, 0:2048], d["d_sohp"].ap()[:, 0:2048])
    nc.sync.dma_start(gp_sb[:, 2048:SG2], d["d_gp"].ap()[:, 2048:SG2])
    nc.sync.dma_start(sohp_sb[:, 2048:SG2], d["d_sohp"].ap()[:, 2048:SG2])
    bc_sb = pW.tile([H, 29], F32, name="bc_sb")
    nc.sync.dma_start(bc_sb[:], d["d_bc"].ap())
    for g in range(1, BL):
        sl = slice(g * SG2, (g + 1) * SG2)
        nc.sync.dma_start(gp_sb[:, sl], d["d_gp"].ap()[:, sl])
        nc.sync.dma_start(sohp_sb[:, sl], d["d_sohp"].ap()[:, sl])
    bh2_sb = pW.tile([OUT, 1], F32, name="bh2_sb")
    nc.sync.dma_start(bh2_sb[:], d["d_bh2"].ap())
    idnb_sb = pW.tile([128, 128], mybir.dt.bfloat16, name="idnb_sb")
    nc.sync.dma_start(idnb_sb[:], d["d_idnb"].ap())
    nc.sync.dma_start(wblob_sb[:, ws1:], d["d_wblob"].ap()[:, ws1:])
    nc.sync.dma_start(wbb_sb[:, bs1:], d["d_wbb"].ap()[:, bs1:])

    if has_empty:
        mrow_sb = pW.tile([1, BL * N], F32, name="mrow_sb")
        nc.sync.dma_start(mrow_sb[:], d["d_mask"].ap())
        nrow_sb = pW.tile([1, BL * N], F32, name="nrow_sb")
        nc.sync.dma_start(nrow_sb[:], d["d_neg"].ap())
        mask_bc = pW.tile([128, BL * N], F32, name="mask_bc")
        nc.gpsimd.partition_broadcast(mask_bc[:], mrow_sb[:])
        neg_bc = pW.tile([128, BL * N], F32, name="neg_bc")
        nc.gpsimd.partition_broadcast(neg_bc[:], nrow_sb[:])

    def W(name):
        off, w = wmap[name]
        return wblob_sb[:, off:off + w]

    def WB(name):
        off, w = bmap[name]
        return wbb_sb[:, off:off + w]

    def pair(ap):
        return ap.rearrange("p (two h) -> p two h", two=2)

    def mov_pair(ap):
        return ap.rearrange("p (w two) -> p two w", two=2)

    # ---- preamble compute (overlaps G DMAs) ------------------------------
    bias_pre = pW.tile([128, L], F32, name="bias_pre")
    nc.vector.tensor_reduce(
        bias_pre[:], bc_sb[:, 0:4 * L].rearrange("p (l f) -> p l f", l=L),
        axis=AX.X, op=ALU.add)
    bo12 = pW.tile([128, L], F32, name="bo12")
    nc.vector.tensor_reduce(
        bo12[:], bc_sb[:, 12:12 + 2 * L].rearrange("p (l f) -> p l f", l=L),
        axis=AX.X, op=ALU.add)

    # node features: 15 fp8 DoubleRow matmuls (5 pairs x 3 levels)
    nf_ps = mps("nf_ps")
    for p in range(5):
        xs = mov_pair(xohp_sb[:, p * XW:(p + 1) * XW])
        for v in range(3):
            blk = (p * 3 + v) * 2 * H
            nc.tensor.matmul(nf_ps[:, 0:BL * N],
                             pair(atomp_sb[:, blk:blk + 2 * H]), xs,
                             start=(p == 0 and v == 0),
                             stop=(p == 4 and v == 2), perf_mode=DR)
    nf = pNM.tile([128, BL * N], BF16, name="nf")
    nc.scalar.activation(nf[:], nf_ps[:, 0:BL * N], AF.Copy)

    def prep_layer(l):
        """bw (bond @ We, xQS basis) + bias_h for layer l; emitted right
        before the layer so chunk-B weight DMAs never block the PE stream."""
        bw_ps = mps("bw_ps")
        nc.tensor.matmul(bw_ps[0:48, 0:H], W("bondT"), W(f"We_{l}"),
                         start=True, stop=True)
        bwp = pW.tile([48, 2 * H], FP8, name=f"bwp{l}")
        nc.scalar.activation(bwp[:, 0:H], bw_ps[0:48, 0:H], AF.Copy, scale=QS)
        nc.vector.scalar_tensor_tensor(bwp[:, H:2 * H], bw_ps[0:48, 0:H], QS,
                                       bwp[:, 0:H], op0=ALU.mult,
                                       op1=ALU.subtract)
        bwp_l[l] = bwp

        bh_ps = mps("bh_ps")
        nc.tensor.matmul(bh_ps[:, 0:2], WB(f"Wo2_{l}"),
                         WB("bp2fb")[:, l:l + 2], start=True, stop=True)
        bias_h = pW.tile([128, 1], F32, name=f"bias_h{l}")
        nc.vector.tensor_tensor(bias_h[:], bh_ps[:, 0:1], bo12[:, l:l + 1],
                                op=ALU.add)
        bias_h_l[l] = bias_h

    bwp_l, bias_h_l = {}, {}
    prep_layer(0)

    # ---- layers (software-pipelined emission) ----------------------------
    ST = {}

    def ensure_state(l):
        if l in ST:
            return
        st = {}
        st["msgs_max"] = pLN.tile([128, BL * N], BF16, name=f"msgs_max{l}",
                                  tag="msgs_max", bufs=2)
        st["h_fm"] = pLN.tile([128, BL * N], BF16, name=f"h_fm{l}",
                              tag="h_fm", bufs=2)
        if l < L - 1:
            st["hid"] = pNM.tile([128, BL * N], BF16, name=f"hid{l}",
                                 tag=f"hid{l}")
        else:
            st["ge_sb"] = pLN.tile([128, BL], BF16, name="ge_sb", tag="ge_sb")
        ST[l] = st

    M12P = {}

    def do_m12(l, gg):
        ensure_state(l)
        gsl = slice(gg * N, (gg + 1) * N)
        ps_m = mps("ps_m")
        nc.tensor.matmul(ps_m[:, 0:2 * H], nf[:, gsl], WB(f"m12_{l}_0"),
                         start=True, stop=(l == 0))
        if l > 0:
            nc.tensor.matmul(ps_m[:, 0:2 * H], ST[l - 1]["hid"][:, gsl],
                             WB(f"m12_{l}_1"), start=False, stop=True)
        m12h = pMB.tile([128, 2 * H], FP8, name=f"m12h{gg}", tag=f"m12h{gg}")
        ew_copy(m12h[:], ps_m[:, 0:2 * H], 2 * H)
        M12P[(l, gg)] = pair(m12h[:])

    def do_tiles(l, gg, t0, t1):
        msgs_max = ST[l]["msgs_max"]
        m12h_p = M12P[(l, gg)]
        bwp_pair = pair(bwp_l[l][:])
        for t in range(t0, t1):
            w = tile_used[t]
            c0 = 2 * (gg * S_graph + t * 512)
            gps = mov_pair(gp_sb[:, c0:c0 + 2 * w])
            sps = mov_pair(sohp_sb[:, c0:c0 + 2 * w])
            pre = ps_pre.tile([128, 512], F32, name="pre")
            nc.tensor.matmul(pre[:, 0:w], m12h_p, gps,
                             start=True, stop=False, perf_mode=DR)
            nc.tensor.matmul(pre[:, 0:w], bwp_pair, sps,
                             start=False, stop=True, perf_mode=DR)
            msgs1 = pAct.tile([128, 512], F32R, name="msgs1", tag="msgs1")
            ew_relu(msgs1[:, 0:w], pre[:, 0:w], bias_pre[:, l:l + 1], w)
            p1 = ps_p1.tile([128, 512], F32, name="p1")
            nc.tensor.matmul(p1[:, 0:w], W(f"Wp1_{l}"), msgs1[:, 0:w],
                             start=True, stop=True)
            msgs2 = pAct.tile([128, 512], F32R, name="msgs2", tag="msgs2")
            ew_relu(msgs2[:, 0:w], p1[:, 0:w], bc_sb[:, 18 + l:19 + l], w)
            p2 = ps_p2.tile([128, 512], F32, name="p2")
            nc.tensor.matmul(p2[:, 0:w], W(f"Wp2_{l}"), msgs2[:, 0:w],
                             start=True, stop=True)
            for (p0, R, K, gt, off) in groups:
                if gt != t:
                    continue
                out_ap = msgs_max[:, gg * N + p0: gg * N + p0 + R]
                seg = p2[:, off:off + R * K].rearrange("p (r k) -> p r k", r=R)
                nc.vector.tensor_reduce(out_ap, seg, axis=AX.X, op=ALU.max)
                load["DVE"] += R * K * 1.042 + OVH["DVE"]

    def do_h(l, gg):
        msgs_max, h_fm = ST[l]["msgs_max"], ST[l]["h_fm"]
        gsl = slice(gg * N, (gg + 1) * N)
        if has_empty:
            mm = pLN.tile([128, N], F32, name="mmx", tag="mmx", bufs=4)
            nc.vector.tensor_tensor(mm[:], msgs_max[:, gsl],
                                    mask_bc[:, gsl], op=ALU.mult)
            nc.vector.tensor_tensor(msgs_max[:, gsl], mm[:],
                                    neg_bc[:, gsl], op=ALU.add)
        h_ps = mps("h_ps")
        nc.tensor.matmul(h_ps[:, 0:N], WB(f"Wo1_{l}_0"), nf[:, gsl],
                         start=True, stop=False)
        if l > 0:
            nc.tensor.matmul(h_ps[:, 0:N], WB(f"Wo1_{l}_1"),
                             ST[l - 1]["hid"][:, gsl], start=False, stop=False)
        nc.tensor.matmul(h_ps[:, 0:N], WB(f"Wo2_{l}"), msgs_max[:, gsl],
                         start=False, stop=True)
        ew_relu(h_fm[:, gsl], h_ps[:, 0:N], bias_h_l[l][:], N)

    def do_ln(l, gg):
        h_fm = ST[l]["h_fm"]
        gsl = slice(gg * N, (gg + 1) * N)
        if l < L - 1:
            # DMA-engine transpose: no PE stall, latency hidden by tiles
            hn = pLN.tile([128, 128], BF16, name="hn", tag="hn", bufs=4)
            nc.sync.dma_start_transpose(hn[:], h_fm[:, gsl])
        else:
            # tail: PE transpose (53ns) keeps the closing chain short
            tp = mps("tp_ps", BF16)
            nc.tensor.transpose(tp[:, 0:128], h_fm[:, gsl], idnb_sb[:])
            hn = tp[:, 0:128]
        st6 = pLN.tile([128, 6], F32, name="st6", tag="st6", bufs=4)
        nc.vector.bn_stats(st6[:], hn)
        charge("DVE", 128)
        st2 = pLN.tile([128, 2], F32, name="st2", tag="st2", bufs=4)
        nc.vector.bn_aggr(st2[:], st6[:])
        charge("DVE", 8)
        std = pLN.tile([128, 1], F32, name="std", tag="std", bufs=4)
        nc.scalar.activation(std[:], st2[:, 1:2], AF.Sqrt,
                             bias=bc_sb[:, 21:22])
        charge("ACT", 1)
        rstd = pLN.tile([128, 1], F32, name="rstd", tag="rstd", bufs=4)
        nc.vector.reciprocal(rstd[:], std[:])
        charge("DVE", 1)
        hnorm = pLN.tile([128, 128], BF16, name="hnorm", tag="hnorm", bufs=4)
        ln_cands = ("DVE", "POOL") if l < L - 1 else ("DVE",)
        ew_ts2(hnorm[:], hn, st2[:, 0:1], rstd[:],
               ALU.subtract, ALU.mult, 128, cands=ln_cands,
               sbuf=(l < L - 1), b2=(l < L - 1))
        if l < L - 1:
            hb = pLN.tile([128, 128], BF16, name="hb", tag="hb", bufs=4)
            nc.sync.dma_start_transpose(hb[:], hnorm[:])
            ew_scale_bias(ST[l]["hid"][:, gsl], hb[:],
                          bc_sb[:, 22 + l:23 + l], bc_sb[:, 25 + l:26 + l],
                          128, sbuf=True)
        else:
            ge_g = mps(f"ge_ps{gg}")
            nc.tensor.matmul(ge_g[:, 0:1], hnorm[:], WB("oneN"),
                             start=True, stop=True)
            nc.scalar.activation(ST[l]["ge_sb"][:, gg:gg + 1], ge_g[:, 0:1],
                                 AF.Identity, scale=bc_sb[:, 24:25],
                                 bias=bc_sb[:, 27:28])
            charge("ACT", 1)

    # pipelined schedule: per-graph h/LN chains are emitted mid-way through
    # the NEXT graph's tile stream (so the PE never waits on trailing
    # reduces), and layer l+1's first graphs overlap layer l's close.
    nsplit = min(3, n_tiles)
    for l in range(L):
        for i in range(BL):
            do_m12(l, i)
            do_tiles(l, i, 0, nsplit)
            if i == 0:
                if l > 0:
                    do_h(l - 1, 3)
                    do_ln(l - 1, 3)
                elif n_tiles > nsplit:
                    pass
            else:
                do_h(l, i - 1)
                do_ln(l, i - 1)
            if i == 2 and l < L - 1:
                prep_layer(l + 1)
            do_tiles(l, i, nsplit, n_tiles)
    do_h(L - 1, 3)
    do_ln(L - 1, 3)
    ge_sb = ST[L - 1]["ge_sb"]

    # ---- head ------------------------------------------------------------
    o1 = mps("o1_ps")
    nc.tensor.matmul(o1[:, 0:BL], WB("Wh1"), ge_sb[:], start=True, stop=True)
    t1 = pLN.tile([128, BL], BF16, name="t1", tag="t1")
    nc.scalar.activation(t1[:], o1[:, 0:BL], AF.Relu, bias=bc_sb[:, 28:29])
    o2 = mps("o2_ps")
    nc.tensor.matmul(o2[:, 0:BL], WB("Wh2"), t1[:], start=True, stop=True)
    out_sb = pLN.tile([OUT, BL], F32, name="out_sb", tag="out_sb")
    nc.scalar.activation(out_sb[:], o2[:, 0:BL], AF.Identity, bias=bh2_sb[:])
    nc.sync.dma_start(d["d_out"].ap(), out_sb[:])
    ctx.close()


# --------------------------------------------------------------------------
# Entry point.
# --------------------------------------------------------------------------

def build(inputs):
    struct, percore = _prep(inputs)
    A = _weight_arrays(inputs)
    wmap = A.pop("_wmap")
    bmap = A.pop("_bmap")
    ws1 = A.pop("_ws1")
    bs1 = A.pop("_bs1")
    key = (struct["S_graph"], struct["n_tiles"],
           tuple(struct["groups"]), struct["has_empty"])
    if key not in _CACHE:
        _CACHE[key] = _build_program(struct, wmap, bmap, ws1, bs1,
                                     A["wblob"].shape[1], A["wbb"].shape[1])
    nc = _CACHE[key]

    in_maps = []
    for c in range(M):
        im = dict(
            gp=percore["gp"][c], sohp=percore["sohp"][c],
            xohp=percore["xohp"][c],
        )
        if struct["has_empty"]:
            im["maskrow"] = percore["maskrow"][c:c + 1]
            im["negrow"] = percore["negrow"][c:c + 1]
        for k, v in A.items():
            im[k] = v
        in_maps.append(im)
    return nc, in_maps, struct


def kernel(**inputs):
    from concourse import bass_utils
    nc, in_maps, struct = build(inputs)
    res = bass_utils.run_bass_kernel_spmd(nc, in_maps, core_ids=list(range(M)))
    out = np.zeros((B, OUT), np.float32)
    for c in range(M):
        out[c * BL:(c + 1) * BL] = res.results[c]["out"].T
    return out
